# revision 27
# baseline (speedup 1.0000x reference)
"""AttentivePoolingNetwork Trainium2 kernel.

B=256 batch sharded 32/core across 8 NeuronCores. Per core:
  Q = cnn_encode(question)   [C=400, 32*40]   (bf16 matmuls, f32 psum)
  A = cnn_encode(answer)     [C=400, 32*400]  (kept in SBUF, bf16)
  P = U^T Q                  [C, 32*40]
  Gpre_b = P_b^T A_b         [40, 400] per batch item (tanh deferred)
  gq[b,m] = max_l Gpre, ga[b,l] = max_m Gpre (tanh applied after max)

C = 400 = 3*128 + 16: the 16-row 4th c-tile is packed 4-wide into PSUM
column strips (tile_position col groups) so its matmuls run concurrently
for 4 batches; G runs 2 batches per PSUM via col groups 0/64.

Softmax over the global batch dim via two AllGathers of the local
exp-sums (batches 0..27 early — latency hidden under phase B — then
28..31) + a PartitionAllReduce over the 16 gathered rows.
exp(tanh(ga)) rows are broadcast (GpSimd partition_broadcast) and
multiplied into A during phase B; post-collective work is the 1/S scale
and a free-dim reduce per c-tile split across DVE and ACT:
  rQ = Q w_q, rA = A w_a, out = cos(rQ, rA)
"""

import numpy as np
import ml_dtypes

import concourse.bass as bass
import concourse.tile as tile
from concourse import bacc, mybir
import concourse.bass_isa as bass_isa
from concourse.bass_utils import run_bass_kernel_spmd

F32 = mybir.dt.float32
BF16 = mybir.dt.bfloat16
AF = mybir.ActivationFunctionType
OP = mybir.AluOpType

N_CORES = 8
B, M, L, E, C = 256, 40, 400, 300, 400
BS = B // N_CORES          # 32 batch per core
CP = 512                   # C padded to 4*128
NT = CP // 128             # 4 c/d tiles
MT = BS * M                # 1280
LT = BS * L                # 12800
CHUNK = 8                  # batch chunk for Q/P matmuls (free dim 320)
NCH = BS // CHUNK          # 4
GRP = 4                    # batch group (e-broadcast + c4 strip packing)
NG = BS // GRP             # 8 groups
PRE = 7                    # xa prefetch depth (batches in flight)
SPLIT = 28                 # batches 0..SPLIT-1 go in the early AllGather

_CACHE = {}


def _build():
    nc = bacc.Bacc("TRN2", target_bir_lowering=False)

    xq_d = nc.dram_tensor("xq", [8, 128, MT], BF16, kind="ExternalInput")
    xa_d = nc.dram_tensor("xa", [BS, 8, 128, 400], BF16, kind="ExternalInput")
    wq_d = nc.dram_tensor("wqt", [8, 128, CP], BF16, kind="ExternalInput")
    wa_d = nc.dram_tensor("wat", [8, 128, CP], BF16, kind="ExternalInput")
    u_d = nc.dram_tensor("ut", [NT, 128, CP], BF16, kind="ExternalInput")
    u3_d = nc.dram_tensor("u3r", [128, CP], BF16, kind="ExternalInput")
    bq_d = nc.dram_tensor("bq", [128, NT], F32, kind="ExternalInput")
    ba_d = nc.dram_tensor("ba", [128, NT], F32, kind="ExternalInput")
    b4_d = nc.dram_tensor("b4r", [128, 2], F32, kind="ExternalInput")
    id_d = nc.dram_tensor("ident", [128, 128], F32, kind="ExternalInput")
    on_d = nc.dram_tensor("ones", [1, 128], BF16, kind="ExternalInput")
    oc_d = nc.dram_tensor("ones_col", [128, 1], F32, kind="ExternalInput")
    out_d = nc.dram_tensor("out", [32], F32, kind="ExternalOutput")

    with tile.TileContext(nc) as tc:
        with tc.tile_pool(name="const", bufs=1) as cp, \
             tc.tile_pool(name="dram", bufs=1, space="DRAM") as dp:
            # ---- persistent SBUF tensors ----
            wa_sb = cp.tile([128, 8 * CP], BF16, tag="wa_sb", name="wa_sb")
            bq_sb = cp.tile([128, NT], F32, tag="bq_sb", name="bq_sb")
            ba_sb = cp.tile([128, NT], F32, tag="ba_sb", name="ba_sb")
            b4_sb = cp.tile([128, 2], F32, tag="b4_sb", name="b4_sb")
            u3_sb = cp.tile([128, CP], BF16, tag="u3_sb", name="u3_sb")
            id_sb = cp.tile([128, 128], F32, tag="id_sb", name="id_sb")
            on_sb = cp.tile([1, 128], BF16, tag="on_sb", name="on_sb")
            oc_sb = cp.tile([128, 1], F32, tag="oc_sb", name="oc_sb")
            # last group's G rows (f32) + transposed-softmax accumulator
            g_s4 = cp.tile([40, 4 * 400], F32, tag="g_s4", name="g_s4")
            et_all = cp.tile([128, 16], F32, tag="et_all", name="et_all")
            # tiles 0..2 full 128 c-rows; tile 3 (c 384:400) strip-packed:
            # q3p strip s = Q4 of chunk s; a3p strip x col-group g = A4 of
            # batch 4g+x
            q_sb = [cp.tile([128, MT], BF16, tag=f"q_sb{t}", name=f"q_sb{t}") for t in range(3)]
            q3p = cp.tile([128, CHUNK * M], BF16, tag="q3p", name="q3p")
            a_sb = [cp.tile([128, LT], BF16, tag=f"a_sb{t}", name=f"a_sb{t}") for t in range(3)]
            a3p = cp.tile([128, NG * 400], BF16, tag="a3p", name="a3p")
            gq_all = cp.tile([40, BS], F32, tag="gq_all", name="gq_all")
            s_acc = [cp.tile([1, 400], F32, tag=f"s_acc{i}", name=f"s_acc{i}") for i in range(2)]
            sq_acc = [cp.tile([40, 1], F32, tag=f"sq_acc{i}", name=f"sq_acc{i}") for i in range(2)]
            rq_t = [cp.tile([128, BS], F32, tag=f"rq{t}", name=f"rq{t}") for t in range(NT)]
            ra_t = [cp.tile([128, BS], F32, tag=f"ra{t}", name=f"ra{t}") for t in range(NT)]
            rq3p = cp.tile([128, CHUNK], F32, tag="rq3p", name="rq3p")
            ra3p = cp.tile([128, NG], F32, tag="ra3p", name="ra3p")

            ccin = [dp.tile([1, 440], F32, tag=f"ccin{i}", name=f"ccin{i}") for i in range(2)]
            ccout = [dp.tile([8, 440], F32, tag=f"ccout{i}", name=f"ccout{i}") for i in range(2)]
            ccwi = dp.tile([1, 8], F32, tag="ccwi", name="ccwi")
            ccwo = dp.tile([1, 8], F32, tag="ccwo", name="ccwo")

            # ---- load constants/inputs ----
            _pab_cm = tc.tile_pool(name="pab", bufs=1)
            pab_pool = _pab_cm.__enter__()
            p_sb = [pab_pool.tile([128, MT], BF16, tag=f"p_sb{t}", name=f"p_sb{t}") for t in range(NT)]
            p4r = pab_pool.tile([128, MT], BF16, tag="p4r", name="p4r")
            _xap_cm = tc.tile_pool(name="xa_pool", bufs=PRE)
            xap = _xap_cm.__enter__()
            _pa_cm = tc.tile_pool(name="pa", bufs=1)
            pa_pool = _pa_cm.__enter__()
            xq_all = pa_pool.tile([128, 8 * MT], BF16, tag="xq_all", name="xq_all")
            wq_sb = pa_pool.tile([128, 8 * CP], BF16, tag="wq_sb", name="wq_sb")
            u_sb = pa_pool.tile([128, NT * CP], BF16, tag="u_sb", name="u_sb")
            # phase-A loads split over the sync and scalar queues so the
            # Q matmuls' inputs all land within the first ~12us
            for k in range(4):
                nc.sync.dma_start(wq_sb[:, k * CP:(k + 1) * CP], wq_d[k])
                nc.scalar.dma_start(wq_sb[:, (k + 4) * CP:(k + 5) * CP], wq_d[k + 4])
            for k in range(4):
                nc.sync.dma_start(xq_all[:, k * MT:k * MT + 640], xq_d[k][:, 0:640])
                nc.scalar.dma_start(xq_all[:, (k + 4) * MT:(k + 4) * MT + 640],
                                    xq_d[k + 4][:, 0:640])
            for k in range(4):
                nc.sync.dma_start(xq_all[:, k * MT + 640:(k + 1) * MT],
                                  xq_d[k][:, 640:MT])
                nc.scalar.dma_start(xq_all[:, (k + 4) * MT + 640:(k + 5) * MT],
                                    xq_d[k + 4][:, 640:MT])
            nc.sync.dma_start(bq_sb[:], bq_d[:])
            nc.scalar.dma_start(
                u_sb[:].rearrange("p (k d) -> p k d", k=NT),
                u_d[:].rearrange("k p d -> p k d"))
            nc.scalar.dma_start(
                wa_sb[:].rearrange("p (k c) -> p k c", k=8),
                wa_d[:].rearrange("k p c -> p k c"))
            nc.scalar.dma_start(ba_sb[:], ba_d[:])
            nc.scalar.dma_start(b4_sb[:], b4_d[:])
            nc.scalar.dma_start(u3_sb[:], u3_d[:])
            nc.sync.dma_start(id_sb[:], id_d[:])
            nc.sync.dma_start(on_sb[:], on_d[:])
            nc.sync.dma_start(oc_sb[:], oc_d[:])

            xts = {}

            def fetch(bb, eng=nc.gpsimd):
                xt = xap.tile([128, 8 * 400], BF16, tag="xa_t", name="xa_t")
                for h in range(2):
                    eng.dma_start(
                        xt[:, 1600 * h:1600 * (h + 1)].rearrange("p (k l) -> p k l", k=4),
                        xa_d[bb][4 * h:4 * (h + 1)].rearrange("k p l -> p k l"))
                xts[bb] = xt

            # stagger the prefetch so it doesn't crowd the phase-A loads
            fetch(0)
            fetch(1)

            # warm the ACT LUT table set (exp_and_others)
            warm = cp.tile([1, 32], F32, tag="warm", name="warm")
            nc.vector.memset(warm[:, :], 0.25)
            nc.scalar.activation(warm[:, :], warm[:, :], AF.Tanh)
            nc.scalar.activation(warm[:, :], warm[:, :], AF.Exp)

            # dummy collective: pre-pays the CC mesh wakeup
            wsync = cp.tile([1, 8], F32, tag="wsync", name="wsync")
            nc.vector.memset(wsync[:, :], 1.0)
            nc.gpsimd.dma_start(ccwi[0:1, :], wsync[:, :])
            nc.gpsimd.collective_compute(
                "AllReduce", OP.add,
                replica_groups=[list(range(N_CORES))],
                ins=[ccwi[:].opt()], outs=[ccwo[:].opt()])

            nc.vector.memset(s_acc[0][:, :], 0.0)
            nc.vector.memset(s_acc[1][:, :], 0.0)
            nc.vector.memset(sq_acc[0][:, :], 0.0)
            nc.vector.memset(sq_acc[1][:, :], 0.0)
            nc.vector.memset(rq_t[3][:, :], 0.0)
            nc.vector.memset(ra_t[3][:, :], 0.0)

            # ---- Phase A: Q encode + P = U^T Q ----
            with tc.tile_pool(name="qpsum", bufs=8, space="PSUM") as qp:
                for t in range(3):
                    ps = [qp.tile([128, CHUNK * M], F32, tag="qps", name="qps") for _ in range(NCH)]
                    for k in range(8):
                        lhsT = wq_sb[:, k * CP + t * 128:k * CP + (t + 1) * 128]
                        for s in range(NCH):
                            rhs = xq_all[:, k * MT + s * CHUNK * M:
                                         k * MT + (s + 1) * CHUNK * M]
                            nc.tensor.matmul(ps[s][:, :], lhsT, rhs,
                                             start=(k == 0), stop=(k == 7))
                    for s in range(NCH):
                        nc.vector.tensor_add(
                            q_sb[t][:, s * CHUNK * M:(s + 1) * CHUNK * M],
                            ps[s][:, :],
                            bq_sb[:, t:t + 1].broadcast_to((128, CHUNK * M)))
                    fetch(2 + t)
                # c4 tile: 4 chunks packed into PSUM col strips 0/32/64/96
                qpk = qp.tile([128, CHUNK * M], F32, tag="qps", name="qpk")
                for k in range(8):
                    lhsT = wq_sb[:, k * CP + 384:k * CP + 400]
                    for s in range(NCH):
                        rhs = xq_all[:, k * MT + s * CHUNK * M:
                                     k * MT + (s + 1) * CHUNK * M]
                        nc.tensor.matmul(qpk[32 * s:32 * s + 16, :], lhsT, rhs,
                                         start=(k == 0), stop=(k == 7),
                                         tile_position=(0, 32 * s))
                nc.scalar.activation(q3p[:, :], qpk[:, :], AF.Identity,
                                     bias=b4_sb[:, 0:1])
                fetch(5)
                fetch(6)

            with tc.tile_pool(name="ppsum", bufs=8, space="PSUM") as pp:
                for t in range(NT):
                    for s in range(NCH):
                        ps = pp.tile([128, CHUNK * M], F32, tag="pps", name="pps")
                        for kk in range(3):
                            lhsT = u_sb[:, kk * CP + t * 128:kk * CP + (t + 1) * 128]
                            rhs = q_sb[kk][:, s * CHUNK * M:(s + 1) * CHUNK * M]
                            nc.tensor.matmul(ps[:, :], lhsT, rhs,
                                             start=(kk == 0), stop=False)
                        # c4 contraction from the strip-packed q3p
                        nc.tensor.matmul(ps[:, :],
                                         u3_sb[32 * s:32 * s + 16, t * 128:(t + 1) * 128],
                                         q3p[32 * s:32 * s + 16, :],
                                         start=False, stop=True,
                                         tile_position=(32 * s, 0))
                        nc.vector.tensor_copy(
                            p_sb[t][:, s * CHUNK * M:(s + 1) * CHUNK * M], ps[:, :])

            _pa_cm.__exit__(None, None, None)

            # replicate P's c4 rows to strips 0/32/64/96 for the packed G kk=3
            for x in range(4):
                nc.sync.dma_start(p4r[32 * x:32 * x + 16, :], p_sb[3][0:16, :])

            # ---- Phase B: per-batch A encode + paired G + maxes + exp ----
            with tc.tile_pool(name="apsum", bufs=4, space="PSUM") as ap, \
                 tc.tile_pool(name="apk", bufs=2, space="PSUM") as apkp, \
                 tc.tile_pool(name="gpsum", bufs=2, space="PSUM") as gp, \
                 tc.tile_pool(name="ebc", bufs=2) as ebcp, \
                 tc.tile_pool(name="rows", bufs=4) as rowp, \
                 tc.tile_pool(name="tree", bufs=4) as trp:

                e_grp = {}
                apk = {}

                g_back_args = {}

                def g_front(bb, g2, off):
                    last = bb >= SPLIT
                    nc.vector.reduce_max(gq_all[0:40, bb:bb + 1], g2[off:off + 40, :],
                                         axis=mybir.AxisListType.X, op=OP.max)
                    e1q = rowp.tile([40, 1], F32, tag="e1q", name="e1q")
                    nc.scalar.activation(e1q[:, :], gq_all[0:40, bb:bb + 1],
                                         AF.Tanh)
                    nc.scalar.activation(e1q[:, :], e1q[:, :], AF.Exp)
                    nc.vector.tensor_add(sq_acc[1 if last else 0][:, :],
                                         sq_acc[1 if last else 0][:, :], e1q[:, :])
                    if last:
                        g_s = g_s4[:, (bb - SPLIT) * 400:(bb - SPLIT + 1) * 400]
                    else:
                        g_s = trp.tile([40, 400], F32, tag="g_s", name="g_s")
                    nc.scalar.activation(g_s[:, :], g2[off:off + 40, :], AF.Copy)
                    g_back_args[bb] = g_s

                def g_back(bb):
                    last = bb >= SPLIT
                    g_s = g_back_args.pop(bb)
                    g_r = trp.tile([40, 400], F32, tag="g_r", name="g_r")
                    nc.gpsimd.partition_all_reduce(
                        g_r[:, :], g_s[:, :], channels=40,
                        reduce_op=bass_isa.ReduceOp.max)
                    t1 = rowp.tile([1, 400], F32, tag="t1", name="t1")
                    nc.scalar.activation(t1[:, :], g_r[0:1, :], AF.Tanh)
                    e1 = rowp.tile([1, 400], F32, tag="e1", name="e1")
                    nc.scalar.activation(e1[:, :], t1[:, :], AF.Exp)
                    if not last:
                        nc.vector.tensor_add(s_acc[0][:, :], s_acc[0][:, :], e1[:, :])
                    e1b = rowp.tile([1, 400], BF16, tag="e1b", name="e1b")
                    nc.vector.tensor_copy(e1b[:, :], e1[:, :])
                    eg = e_grp[bb // GRP]
                    nc.gpsimd.partition_broadcast(
                        eg[:, (bb % GRP) * 400:(bb % GRP + 1) * 400], e1b[:, :])

                def g_chain(bb, g2, off):
                    g_front(bb, g2, off)
                    g_back(bb)

                g_pair_ps = {}

                def do_g_pair_mm(b0):
                    """G matmuls only; g2 psum kept for deferred fronts."""
                    g2 = gp.tile([128, 400], F32, tag="gps", name="gps")
                    for kk in range(3):
                        for j in range(2):
                            bb = b0 + j
                            nc.tensor.matmul(
                                g2[64 * j:64 * j + 40, :],
                                p_sb[kk][:, bb * M:(bb + 1) * M],
                                a_sb[kk][:, bb * L:(bb + 1) * L],
                                start=(kk == 0), stop=False,
                                tile_position=(0, 64 * j))
                    for j in range(2):
                        bb = b0 + j
                        x = bb % GRP
                        g = bb // GRP
                        nc.tensor.matmul(
                            g2[64 * j:64 * j + 40, :],
                            p4r[32 * x:32 * x + 16, bb * M:(bb + 1) * M],
                            a3p[32 * x:32 * x + 16, g * 400:(g + 1) * 400],
                            start=False, stop=(j == 1),
                            tile_position=(32 * x, 64 * j))
                    g_pair_ps[b0] = g2

                def do_g_pair(b0):
                    """G for batches b0, b0+1 packed via col groups 0/64."""
                    g2 = gp.tile([128, 400], F32, tag="gps", name="gps")
                    for kk in range(3):
                        for j in range(2):
                            bb = b0 + j
                            nc.tensor.matmul(
                                g2[64 * j:64 * j + 40, :],
                                p_sb[kk][:, bb * M:(bb + 1) * M],
                                a_sb[kk][:, bb * L:(bb + 1) * L],
                                start=(kk == 0), stop=False,
                                tile_position=(0, 64 * j))
                    for j in range(2):
                        bb = b0 + j
                        x = bb % GRP
                        g = bb // GRP
                        nc.tensor.matmul(
                            g2[64 * j:64 * j + 40, :],
                            p4r[32 * x:32 * x + 16, bb * M:(bb + 1) * M],
                            a3p[32 * x:32 * x + 16, g * 400:(g + 1) * 400],
                            start=False, stop=(j == 1),
                            tile_position=(32 * x, 64 * j))
                    g_chain(b0, g2, 0)
                    g_chain(b0 + 1, g2, 64)

                def y_mult(g):
                    # Y = A .* exp-broadcast over this group's slice
                    eg = e_grp.pop(g)
                    sl = slice(g * GRP * L, (g + 1) * GRP * L)
                    for t in range(3):
                        nc.vector.tensor_tensor(a_sb[t][:, sl], a_sb[t][:, sl],
                                                eg[:, :], op=OP.mult)
                    # strip-packed c4 tile: per-strip slices of eg line up
                    # with the per-batch strips of a3p
                    eg3 = ebcp.tile([128, 400], BF16, tag="eg3", name="eg3")
                    for x in range(GRP):
                        nc.vector.tensor_copy(
                            eg3[32 * x:32 * x + 16, :],
                            eg[32 * x:32 * x + 16, x * 400:(x + 1) * 400])
                    nc.vector.tensor_tensor(a3p[:, g * 400:(g + 1) * 400],
                                            a3p[:, g * 400:(g + 1) * 400],
                                            eg3[:, :], op=OP.mult)

                for bb in range(BS):
                    if bb + PRE < BS:
                        fetch(bb + PRE)
                    g = bb // GRP
                    x = bb % GRP
                    if x == 0:
                        e_grp[g] = ebcp.tile([128, GRP * 400], BF16,
                                             tag="e_g", name="e_g")
                        apk[g] = apkp.tile([128, 400], F32, tag="apk", name="apk")
                    xt = xts[bb]
                    for t in range(3):
                        aps = ap.tile([128, 400], F32, tag="aps", name="aps")
                        for k in range(8):
                            lhsT = wa_sb[:, k * CP + t * 128:k * CP + (t + 1) * 128]
                            nc.tensor.matmul(aps[:, :], lhsT, xt[:, k * 400:(k + 1) * 400],
                                             start=(k == 0), stop=(k == 7))
                        nc.scalar.activation(a_sb[t][:, bb * L:(bb + 1) * L],
                                             aps[:, :], AF.Identity,
                                             bias=ba_sb[:, t:t + 1])
                    if x == GRP - 1:
                        # c4 rows for the whole group: k-outer / strip-inner
                        # so the 4 col-group matmuls of each k overlap in
                        # the PE array
                        pk = apk.pop(g)
                        for k in range(8):
                            lhsT = wa_sb[:, k * CP + 384:k * CP + 400]
                            for x2 in range(GRP):
                                xt2 = xts[bb - 3 + x2]
                                nc.tensor.matmul(pk[32 * x2:32 * x2 + 16, :], lhsT,
                                                 xt2[:, k * 400:(k + 1) * 400],
                                                 start=(k == 0), stop=(k == 7),
                                                 tile_position=(0, 32 * x2))
                        nc.scalar.activation(a3p[:, g * 400:(g + 1) * 400],
                                             pk[:, :], AF.Identity,
                                             bias=b4_sb[:, 1:2])
                        for x2 in range(GRP):
                            xts.pop(bb - 3 + x2)
                        if bb < BS - 1:
                            do_g_pair(bb - 3)
                            do_g_pair(bb - 1)
                            y_mult(g)
                        else:
                            do_g_pair_mm(bb - 3)
                            do_g_pair_mm(bb - 1)
                            for b2 in range(SPLIT, BS):
                                g_front(b2, g_pair_ps[SPLIT + 2 * ((b2 - SPLIT) // 2)],
                                        64 * ((b2 - SPLIT) % 2))
                            # transposed AllGather-2 contribution on the now
                            # idle PE; its trigger goes ahead of the gpsimd
                            # PAR/broadcast tail
                            for q2 in range(4):
                                tpb = gp.tile([128, 400], F32, tag="gps", name="tpb")
                                for j in range(4):
                                    w = 128 if j < 3 else 16
                                    nc.tensor.transpose(
                                        tpb[0:w, 40 * j:40 * j + 40],
                                        g_s4[:, q2 * 400 + 128 * j:q2 * 400 + 128 * j + w],
                                        id_sb[0:40, 0:40])
                                nc.vector.reduce_max(
                                    et_all[:, 4 * q2:4 * q2 + 4],
                                    tpb[:, 0:160].rearrange("p (j m) -> p j m", j=4),
                                    axis=mybir.AxisListType.X, op=OP.max)
                            nc.scalar.activation(et_all[:, :], et_all[:, :], AF.Tanh)
                            nc.scalar.activation(et_all[:, :], et_all[:, :], AF.Exp)
                            et_v = et_all[:].rearrange("p (q j) -> p q j", q=4)
                            nc.vector.tensor_tensor(et_v[:, 0:2, :], et_v[:, 0:2, :],
                                                    et_v[:, 2:4, :], op=OP.add)
                            sT = rowp.tile([128, 4], F32, tag="sT", name="sT")
                            nc.vector.tensor_tensor(
                                sT[:, :].rearrange("p (o j) -> p o j", o=1),
                                et_v[:, 0:1, :], et_v[:, 1:2, :], op=OP.add)
                            nc.sync.dma_start(ccin[1][0:1, 0:40], sq_acc[1][:, :])
                            nc.sync.dma_start(
                                ccin[1][0:1, 40:424].rearrange("o (j i) -> i (o j)", j=3),
                                sT[:, 0:3])
                            nc.sync.dma_start(
                                ccin[1][0:1, 424:440].rearrange("o (j i) -> i (o j)", j=1),
                                sT[0:16, 3:4])
                            nc.gpsimd.collective_compute(
                                "AllGather", OP.bypass,
                                replica_groups=[list(range(N_CORES))],
                                ins=[ccin[1][:].opt()], outs=[ccout[1][:].opt()])
                            for b2 in range(SPLIT, BS):
                                g_back(b2)
                            y_mult(g)
                    if bb == SPLIT - 1:
                        nc.sync.dma_start(ccin[0][0:1, 0:40], sq_acc[0][:, :])
                        nc.sync.dma_start(ccin[0][0:1, 40:440], s_acc[0][:, :])
                        nc.gpsimd.collective_compute(
                            "AllGather", OP.bypass,
                            replica_groups=[list(range(N_CORES))],
                            ins=[ccin[0][:].opt()], outs=[ccout[0][:].opt()])

            _xap_cm.__exit__(None, None, None)
            _pab_cm.__exit__(None, None, None)

            # ---- Phase C: AllGather #2 + 1/S scale + pooled sums ----
            with tc.tile_pool(name="phc", bufs=1) as pc, \
                 tc.tile_pool(name="cpsum", bufs=2, space="PSUM") as cps, \
                 tc.tile_pool(name="cpsum1", bufs=2, space="PSUM") as cp1:
                sg = pc.tile([16, 440], F32, tag="sg", name="sg")
                nc.sync.dma_start(sg[0:8, :], ccout[0][:, :])

                # --- work hidden under the collective: exp_q ---
                tq = pc.tile([40, BS], F32, tag="tq", name="tq")
                nc.scalar.activation(tq[:, :], gq_all[:, :], AF.Tanh)
                e_q = pc.tile([40, BS], F32, tag="e_q", name="e_q")
                nc.scalar.activation(e_q[:, :], tq[:, :], AF.Exp)
                eqt_ps = cp1.tile([BS, 40], F32, tag="c1", name="eqt_ps", bufs=1)
                nc.tensor.transpose(eqt_ps[:, :], e_q[:, :], id_sb[0:40, 0:40])
                eqt = pc.tile([BS, 40], BF16, tag="eqt", name="eqt")
                nc.vector.tensor_copy(eqt[:, :], eqt_ps[:, :])
                eq_fl = pc.tile([1, MT], BF16, tag="eq_fl", name="eq_fl")
                nc.gpsimd.dma_start(eq_fl[0:1, :], eqt[:, :])
                eq_bc = pc.tile([128, MT], BF16, tag="eq_bc", name="eq_bc")
                for ch in range(0, MT, 512):
                    ce = min(ch + 512, MT)
                    wqb = cps.tile([128, 512], F32, tag="wqb", name="wqb", bufs=1)
                    nc.tensor.matmul(wqb[:, 0:ce - ch], on_sb[:, :],
                                     eq_fl[0:1, ch:ce], start=True, stop=True)
                    nc.scalar.activation(eq_bc[:, ch:ce], wqb[:, 0:ce - ch], AF.Copy)
                for t in range(3):
                    nc.vector.tensor_tensor(q_sb[t][:, :], q_sb[t][:, :],
                                            eq_bc[:, :], op=OP.mult)
                eq3 = pc.tile([128, CHUNK * M], BF16, tag="eq3", name="eq3")
                for sx in range(NCH):
                    nc.vector.tensor_copy(
                        eq3[32 * sx:32 * sx + 16, :],
                        eq_bc[32 * sx:32 * sx + 16, sx * CHUNK * M:(sx + 1) * CHUNK * M])
                nc.vector.tensor_tensor(q3p[:, :], q3p[:, :], eq3[:, :], op=OP.mult)

                # --- post-collective: u = 1/S ---
                nc.sync.dma_start(sg[8:16, :], ccout[1][:, :])
                sgp = cp1.tile([1, 440], F32, tag="sgp", name="sgp", bufs=1)
                nc.tensor.matmul(sgp[:, :], oc_sb[0:16, 0:1], sg[:, :],
                                 start=True, stop=True)
                ur = pc.tile([1, 440], F32, tag="ur", name="ur")
                nc.vector.reciprocal_approx_fast(ur[:, :], sgp[0:1, :])
                ub = pc.tile([1, 440], BF16, tag="ub", name="ub")
                nc.vector.tensor_copy(ub[:, :], ur[:, :])
                ua_bc = pc.tile([128, 400], BF16, tag="ua_bc", name="ua_bc")
                nc.gpsimd.partition_broadcast(ua_bc[:, :], ub[0:1, 40:440])

                uq_bc = pc.tile([128, 40], BF16, tag="uq_bc", name="uq_bc")
                nc.gpsimd.partition_broadcast(uq_bc[:, :], ub[0:1, 0:40])
                ua_v = ua_bc[:].rearrange("p (o l) -> p o l", o=1).broadcast_to((128, BS, 400))
                ua_v8 = ua_bc[:].rearrange("p (o l) -> p o l", o=1).broadcast_to((128, NG, 400))
                uq_v = uq_bc[:].rearrange("p (o m) -> p o m", o=1).broadcast_to((128, BS, 40))
                uq_v8 = uq_bc[:].rearrange("p (o m) -> p o m", o=1).broadcast_to((128, CHUNK, 40))

                def tree_sum(av, out, n, nb):
                    while n > 25 and n % 2 == 0:
                        h = n // 2
                        nc.vector.tensor_tensor(av[:, 0:nb, 0:h], av[:, 0:nb, 0:h],
                                                av[:, 0:nb, h:n], op=OP.add)
                        n = h
                    nc.vector.reduce_sum(out, av[:, 0:nb, 0:n],
                                         axis=mybir.AxisListType.X, op=OP.add)

                # c4 tile first: small, unblocks the repack DMA early
                a3v = a3p[:].rearrange("p (g l) -> p g l", g=NG)
                nc.vector.tensor_tensor(a3v, a3v, ua_v8, op=OP.mult)
                tree_sum(a3v, ra3p[:, :], 400, NG)
                # repack [strip x | c16, g] -> [c16, b=4g+x]
                for x in range(4):
                    nc.sync.dma_start(
                        ra_t[3][0:16, :].rearrange("i (g x) -> i x g", x=4)[:, x:x + 1, :],
                        ra3p[32 * x:32 * x + 16, :].rearrange("i (o g) -> i o g", o=1))
                scr = pc.tile([128, 400], BF16, tag="scr", name="scr")
                for t in range(3):
                    av = a_sb[t][:].rearrange("p (b l) -> p b l", b=BS)
                    nc.vector.tensor_tensor(av, av, ua_v, op=OP.mult)
                    if t == 0:
                        # otherwise-idle scalar engine reduces one c-tile
                        for bb in range(BS):
                            nc.scalar.activation(
                                scr[:, :], av[:, bb, :], AF.Copy,
                                accum_out=ra_t[t][:, bb:bb + 1])
                    elif t == 1:
                        for bb in range(8):
                            nc.scalar.activation(
                                scr[:, :], av[:, bb, :], AF.Copy,
                                accum_out=ra_t[t][:, bb:bb + 1])
                        tv = av[:, 8:BS, :]
                        n = 400
                        while n > 25 and n % 2 == 0:
                            h = n // 2
                            nc.vector.tensor_tensor(tv[:, :, 0:h], tv[:, :, 0:h],
                                                    tv[:, :, h:n], op=OP.add)
                            n = h
                        nc.vector.reduce_sum(ra_t[t][:, 8:BS], tv[:, :, 0:n],
                                             axis=mybir.AxisListType.X, op=OP.add)
                    else:
                        tree_sum(av, ra_t[t][:, :], 400, BS)
                q3v = q3p[:].rearrange("p (j m) -> p j m", j=CHUNK)
                nc.vector.tensor_tensor(q3v, q3v, uq_v8, op=OP.mult)
                tree_sum(q3v, rq3p[:, :], 40, CHUNK)
                for x in range(4):
                    nc.sync.dma_start(
                        rq_t[3][0:16, :].rearrange("i (s j) -> i s j", s=4)[:, x:x + 1, :],
                        rq3p[32 * x:32 * x + 16, :].rearrange("i (o j) -> i o j", o=1))

                for t in range(3):
                    qv = q_sb[t][:].rearrange("p (b m) -> p b m", b=BS)
                    nc.vector.tensor_tensor(qv, qv, uq_v, op=OP.mult)
                    tree_sum(qv, rq_t[t][:, :], 40, BS)

                # cosine similarity via accumulating PE transposes
                def psum_all(tiles, tag):
                    tps = cps.tile([BS, 128], F32, tag="cts", name=f"{tag}tp", bufs=2)
                    for t in range(NT):
                        nc.tensor.matmul(tps[:, :], tiles[t][:, :], id_sb[:, :],
                                         is_transpose=True,
                                         start=(t == 0), stop=(t == NT - 1))
                    col = pc.tile([32, 1], F32, tag=f"{tag}c", name=f"{tag}c")
                    nc.vector.reduce_sum(col[:, :], tps[:, :],
                                         axis=mybir.AxisListType.X, op=OP.add)
                    return col

                pr = [pc.tile([128, BS], F32, tag=f"pr{t}", name=f"pr{t}") for t in range(NT)]
                pq = [pc.tile([128, BS], F32, tag=f"pq{t}", name=f"pq{t}") for t in range(NT)]
                pa = [pc.tile([128, BS], F32, tag=f"pa{t}", name=f"pa{t}") for t in range(NT)]
                for t in range(NT):
                    nc.vector.tensor_mul(pr[t][:, :], rq_t[t][:, :], ra_t[t][:, :])
                    nc.vector.tensor_mul(pq[t][:, :], rq_t[t][:, :], rq_t[t][:, :])
                    nc.vector.tensor_mul(pa[t][:, :], ra_t[t][:, :], ra_t[t][:, :])
                dot = psum_all(pr, "dt")
                qq = psum_all(pq, "qq")
                aa = psum_all(pa, "aa")

                nq = pc.tile([32, 1], F32, tag="nq", name="nq")
                na = pc.tile([32, 1], F32, tag="na", name="na")
                nc.scalar.activation(nq[:, :], qq[:, :], AF.Sqrt)
                nc.scalar.activation(na[:, :], aa[:, :], AF.Sqrt)
                nc.vector.tensor_scalar_max(nq[:, :], nq[:, :], 1e-6)
                nc.vector.tensor_scalar_max(na[:, :], na[:, :], 1e-6)
                den = pc.tile([32, 1], F32, tag="den", name="den")
                nc.vector.tensor_mul(den[:, :], nq[:, :], na[:, :])
                rden = pc.tile([32, 1], F32, tag="rden", name="rden")
                nc.vector.reciprocal(rden[:, :], den[:, :])
                res = pc.tile([32, 1], F32, tag="res", name="res")
                nc.vector.tensor_mul(res[:, :], dot[:, :], rden[:, :])
                nc.gpsimd.dma_start(out_d[:].rearrange("(a b) -> a b", b=1),
                                    res[:, :])

    nc.finalize()
    return nc


def _prep(question, answer, Wq, bq, Wa, ba, U):
    bf = ml_dtypes.bfloat16
    qs = question.reshape(N_CORES, BS, M, E)
    as_ = answer.reshape(N_CORES, BS, L, E)

    def enc_z8(x, T):
        # x: [BS, T, E] -> Z^T rows [BS, 8, 128, T] bf16 (ctx shifts baked in)
        xt = x.transpose(0, 2, 1)  # [BS, E, T]
        xtp = np.zeros((x.shape[0], E, T + 2), np.float32)
        xtp[:, :, 1:T + 1] = xt
        z = np.zeros((x.shape[0], 1024, T), dtype=bf)
        for i in range(3):
            z[:, i * E:(i + 1) * E, :] = xtp[:, :, i:i + T].astype(bf)
        return z.reshape(x.shape[0], 8, 128, T)

    def enc_xq8(x):
        # [BS, M, E] -> [8, 128, BS*M] bf16
        z = enc_z8(x, M)  # [BS, 8, 128, 40]
        return np.ascontiguousarray(z.transpose(1, 2, 0, 3)).reshape(8, 128, MT)

    def enc_w8(W):
        # W [C, 900] -> W^T padded [8, 128, CP] bf16
        o = np.zeros((1024, CP), dtype=bf)
        o[0:900, 0:C] = W.T.astype(bf)
        return o.reshape(8, 128, CP)

    up = np.zeros((CP, CP), dtype=bf)
    up[0:C, 0:C] = U.astype(bf)
    up = up.reshape(NT, 128, CP)

    # U rows 384:400 replicated at partition strips 0/32/64/96
    u3r = np.zeros((128, CP), dtype=bf)
    for x in range(4):
        u3r[32 * x:32 * x + 16, 0:C] = U[384:400, :].astype(bf)

    def enc_b(b):
        o = np.zeros((CP,), np.float32)
        o[0:C] = b
        return np.ascontiguousarray(o.reshape(NT, 128).T)

    # bias rows 384:400 replicated at strips, for the packed c4 drains
    b4r = np.zeros((128, 2), np.float32)
    for x in range(4):
        b4r[32 * x:32 * x + 16, 0] = bq[384:400]
        b4r[32 * x:32 * x + 16, 1] = ba[384:400]

    com = {
        "wqt": enc_w8(Wq), "wat": enc_w8(Wa), "ut": up, "u3r": u3r,
        "bq": enc_b(bq), "ba": enc_b(ba), "b4r": b4r,
        "ident": np.eye(128, dtype=np.float32),
        "ones": np.ones((1, 128), dtype=bf),
        "ones_col": np.ones((128, 1), np.float32),
    }
    maps = []
    for i in range(N_CORES):
        m = dict(com)
        m["xq"] = enc_xq8(qs[i])
        m["xa"] = enc_z8(as_[i], L)
        maps.append(m)
    return maps


def kernel(question, answer, Wq, bq, Wa, ba, U, _trace=False):
    if "nc" not in _CACHE:
        _CACHE["nc"] = _build()
    nc = _CACHE["nc"]
    maps = _prep(np.asarray(question), np.asarray(answer), np.asarray(Wq),
                 np.asarray(bq), np.asarray(Wa), np.asarray(ba), np.asarray(U))
    r = run_bass_kernel_spmd(nc, maps, list(range(N_CORES)), trace=_trace)
    _CACHE["last"] = r
    return np.concatenate([r.results[i]["out"] for i in range(N_CORES)])


# revision 29
# speedup vs baseline: 1.1189x; 1.1189x over previous
"""AttentivePoolingNetwork Trainium2 kernel.

B=256 batch sharded 32/core across 8 NeuronCores. Per core:
  Q = cnn_encode(question)   [C=400, 32*40]   (bf16 matmuls, f32 psum)
  A = cnn_encode(answer)     [C=400, 32*400]  (kept in SBUF, bf16)
  P = U^T Q                  [C, 32*40]
  Gpre_b = P_b^T A_b         [40, 400] per batch item (tanh deferred)
  gq[b,m] = max_l Gpre, ga[b,l] = max_m Gpre (tanh applied after max)

C = 400 = 3*128 + 16: the 16-row 4th c-tile is packed 4-wide into PSUM
column strips (tile_position col groups) so its matmuls run concurrently
for 4 batches; G runs 2 batches per PSUM via col groups 0/64.

Softmax over the global batch dim via two AllGathers of the local
exp-sums (batches 0..27 early — latency hidden under phase B — then
28..31) + a PartitionAllReduce over the 16 gathered rows.
exp(tanh(ga)) rows are broadcast (GpSimd partition_broadcast) and
multiplied into A during phase B; post-collective work is the 1/S scale
and a free-dim reduce per c-tile split across DVE and ACT:
  rQ = Q w_q, rA = A w_a, out = cos(rQ, rA)
"""

import numpy as np
import ml_dtypes

import concourse.bass as bass
import concourse.tile as tile
from concourse import bacc, mybir
import concourse.bass_isa as bass_isa
from concourse.bass_utils import run_bass_kernel_spmd

F32 = mybir.dt.float32
BF16 = mybir.dt.bfloat16
AF = mybir.ActivationFunctionType
OP = mybir.AluOpType

N_CORES = 8
B, M, L, E, C = 256, 40, 400, 300, 400
BS = B // N_CORES          # 32 batch per core
CP = 512                   # C padded to 4*128
NT = CP // 128             # 4 c/d tiles
MT = BS * M                # 1280
LT = BS * L                # 12800
CHUNK = 8                  # batch chunk for Q/P matmuls (free dim 320)
NCH = BS // CHUNK          # 4
GRP = 4                    # batch group (e-broadcast + c4 strip packing)
NG = BS // GRP             # 8 groups
PRE = 6                    # xa prefetch depth (batches in flight)
SPLIT = 28                 # batches 0..SPLIT-1 go in the early AllGather

_CACHE = {}


def _build():
    nc = bacc.Bacc("TRN2", target_bir_lowering=False)

    xq_d = nc.dram_tensor("xq", [8, 128, MT], BF16, kind="ExternalInput")
    xa_d = nc.dram_tensor("xa", [BS, 8, 128, 400], BF16, kind="ExternalInput")
    wq_d = nc.dram_tensor("wqt", [8, 128, CP], BF16, kind="ExternalInput")
    wa_d = nc.dram_tensor("wat", [8, 128, CP], BF16, kind="ExternalInput")
    u_d = nc.dram_tensor("ut", [NT, 128, CP], BF16, kind="ExternalInput")
    u3_d = nc.dram_tensor("u3r", [128, CP], BF16, kind="ExternalInput")
    bq_d = nc.dram_tensor("bq", [128, NT], F32, kind="ExternalInput")
    ba_d = nc.dram_tensor("ba", [128, NT], F32, kind="ExternalInput")
    b4_d = nc.dram_tensor("b4r", [128, 2], F32, kind="ExternalInput")
    id_d = nc.dram_tensor("ident", [128, 128], F32, kind="ExternalInput")
    on_d = nc.dram_tensor("ones", [1, 128], BF16, kind="ExternalInput")
    oc_d = nc.dram_tensor("ones_col", [128, 1], F32, kind="ExternalInput")
    out_d = nc.dram_tensor("out", [32], F32, kind="ExternalOutput")

    with tile.TileContext(nc) as tc:
        with tc.tile_pool(name="const", bufs=1) as cp, \
             tc.tile_pool(name="dram", bufs=1, space="DRAM") as dp:
            # ---- persistent SBUF tensors ----
            wa_sb = cp.tile([128, 8 * CP], BF16, tag="wa_sb", name="wa_sb")
            bq_sb = cp.tile([128, NT], F32, tag="bq_sb", name="bq_sb")
            ba_sb = cp.tile([128, NT], F32, tag="ba_sb", name="ba_sb")
            b4_sb = cp.tile([128, 2], F32, tag="b4_sb", name="b4_sb")
            u3_sb = cp.tile([128, CP], BF16, tag="u3_sb", name="u3_sb")
            id_sb = cp.tile([128, 128], F32, tag="id_sb", name="id_sb")
            on_sb = cp.tile([1, 128], BF16, tag="on_sb", name="on_sb")
            oc_sb = cp.tile([128, 1], F32, tag="oc_sb", name="oc_sb")
            # last group's G rows (f32) + transposed-softmax accumulator
            g_s4 = cp.tile([40, 4 * 400], F32, tag="g_s4", name="g_s4")
            et_all = cp.tile([128, 16], F32, tag="et_all", name="et_all")
            # tiles 0..2 full 128 c-rows; tile 3 (c 384:400) strip-packed:
            # q3p strip s = Q4 of chunk s; a3p strip x col-group g = A4 of
            # batch 4g+x
            q_sb = [cp.tile([128, MT], BF16, tag=f"q_sb{t}", name=f"q_sb{t}") for t in range(3)]
            q3p = cp.tile([128, CHUNK * M], BF16, tag="q3p", name="q3p")
            a_sb = [cp.tile([128, LT], BF16, tag=f"a_sb{t}", name=f"a_sb{t}") for t in range(3)]
            a3p = cp.tile([128, NG * 400], BF16, tag="a3p", name="a3p")
            gq_all = cp.tile([40, BS], F32, tag="gq_all", name="gq_all")
            s_acc = [cp.tile([1, 400], F32, tag=f"s_acc{i}", name=f"s_acc{i}") for i in range(2)]
            sq_acc = [cp.tile([40, 1], F32, tag=f"sq_acc{i}", name=f"sq_acc{i}") for i in range(2)]
            rq_t = [cp.tile([128, BS], F32, tag=f"rq{t}", name=f"rq{t}") for t in range(NT)]
            ra_t = [cp.tile([128, BS], F32, tag=f"ra{t}", name=f"ra{t}") for t in range(NT)]
            rq3p = cp.tile([128, CHUNK], F32, tag="rq3p", name="rq3p")
            ra3p = cp.tile([128, NG], F32, tag="ra3p", name="ra3p")

            ccin = [dp.tile([1, 440], F32, tag=f"ccin{i}", name=f"ccin{i}") for i in range(2)]
            ccout = [dp.tile([8, 440], F32, tag=f"ccout{i}", name=f"ccout{i}") for i in range(2)]
            ccwi = dp.tile([1, 8], F32, tag="ccwi", name="ccwi")
            ccwo = dp.tile([1, 8], F32, tag="ccwo", name="ccwo")

            # ---- load constants/inputs ----
            _pab_cm = tc.tile_pool(name="pab", bufs=1)
            pab_pool = _pab_cm.__enter__()
            p_sb = [pab_pool.tile([128, MT], BF16, tag=f"p_sb{t}", name=f"p_sb{t}") for t in range(NT)]
            p4r = pab_pool.tile([128, MT], BF16, tag="p4r", name="p4r")
            _xap_cm = tc.tile_pool(name="xa_pool", bufs=PRE)
            xap = _xap_cm.__enter__()
            _pa_cm = tc.tile_pool(name="pa", bufs=1)
            pa_pool = _pa_cm.__enter__()
            xq_all = pa_pool.tile([128, 8 * MT], BF16, tag="xq_all", name="xq_all")
            wq_sb = pa_pool.tile([128, 8 * CP], BF16, tag="wq_sb", name="wq_sb")
            u_sb = pa_pool.tile([128, NT * CP], BF16, tag="u_sb", name="u_sb")
            # phase-A loads split over the sync and scalar queues so the
            # Q matmuls' inputs all land within the first ~12us
            for k in range(4):
                nc.sync.dma_start(wq_sb[:, k * CP:(k + 1) * CP], wq_d[k])
                nc.scalar.dma_start(wq_sb[:, (k + 4) * CP:(k + 5) * CP], wq_d[k + 4])
            for k in range(4):
                nc.sync.dma_start(xq_all[:, k * MT:k * MT + 640], xq_d[k][:, 0:640])
                nc.scalar.dma_start(xq_all[:, (k + 4) * MT:(k + 4) * MT + 640],
                                    xq_d[k + 4][:, 0:640])
            for k in range(4):
                nc.sync.dma_start(xq_all[:, k * MT + 640:(k + 1) * MT],
                                  xq_d[k][:, 640:MT])
                nc.scalar.dma_start(xq_all[:, (k + 4) * MT + 640:(k + 5) * MT],
                                    xq_d[k + 4][:, 640:MT])
            nc.sync.dma_start(bq_sb[:], bq_d[:])
            nc.scalar.dma_start(
                u_sb[:].rearrange("p (k d) -> p k d", k=NT),
                u_d[:].rearrange("k p d -> p k d"))
            nc.scalar.dma_start(
                wa_sb[:].rearrange("p (k c) -> p k c", k=8),
                wa_d[:].rearrange("k p c -> p k c"))
            nc.scalar.dma_start(ba_sb[:], ba_d[:])
            nc.scalar.dma_start(b4_sb[:], b4_d[:])
            nc.scalar.dma_start(u3_sb[:], u3_d[:])
            nc.sync.dma_start(id_sb[:], id_d[:])
            nc.sync.dma_start(on_sb[:], on_d[:])
            nc.sync.dma_start(oc_sb[:], oc_d[:])

            xts = {}

            def fetch(bb, eng=nc.gpsimd):
                xt = xap.tile([128, 8 * 400], BF16, tag="xa_t", name="xa_t")
                for h in range(2):
                    eng.dma_start(
                        xt[:, 1600 * h:1600 * (h + 1)].rearrange("p (k l) -> p k l", k=4),
                        xa_d[bb][4 * h:4 * (h + 1)].rearrange("k p l -> p k l"))
                xts[bb] = xt

            # stagger the prefetch so it doesn't crowd the phase-A loads
            fetch(0)
            fetch(1)

            # warm the ACT LUT table set (exp_and_others)
            warm = cp.tile([1, 32], F32, tag="warm", name="warm")
            nc.vector.memset(warm[:, :], 0.25)
            nc.scalar.activation(warm[:, :], warm[:, :], AF.Tanh)
            nc.scalar.activation(warm[:, :], warm[:, :], AF.Exp)

            # dummy collective: pre-pays the CC mesh wakeup
            wsync = cp.tile([1, 8], F32, tag="wsync", name="wsync")
            nc.vector.memset(wsync[:, :], 1.0)
            nc.gpsimd.dma_start(ccwi[0:1, :], wsync[:, :])
            nc.gpsimd.collective_compute(
                "AllReduce", OP.add,
                replica_groups=[list(range(N_CORES))],
                ins=[ccwi[:].opt()], outs=[ccwo[:].opt()])

            nc.vector.memset(s_acc[0][:, :], 0.0)
            nc.vector.memset(s_acc[1][:, :], 0.0)
            nc.vector.memset(sq_acc[0][:, :], 0.0)
            nc.vector.memset(sq_acc[1][:, :], 0.0)
            nc.vector.memset(rq_t[3][:, :], 0.0)
            nc.vector.memset(ra_t[3][:, :], 0.0)

            # ---- Phase A: Q encode + P = U^T Q ----
            with tc.tile_pool(name="qpsum", bufs=8, space="PSUM") as qp:
                for t in range(3):
                    ps = [qp.tile([128, CHUNK * M], F32, tag="qps", name="qps") for _ in range(NCH)]
                    for k in range(8):
                        lhsT = wq_sb[:, k * CP + t * 128:k * CP + (t + 1) * 128]
                        for s in range(NCH):
                            rhs = xq_all[:, k * MT + s * CHUNK * M:
                                         k * MT + (s + 1) * CHUNK * M]
                            nc.tensor.matmul(ps[s][:, :], lhsT, rhs,
                                             start=(k == 0), stop=(k == 7))
                    for s in range(NCH):
                        nc.vector.tensor_add(
                            q_sb[t][:, s * CHUNK * M:(s + 1) * CHUNK * M],
                            ps[s][:, :],
                            bq_sb[:, t:t + 1].broadcast_to((128, CHUNK * M)))
                    fetch(2 + t)
                # c4 tile: 4 chunks packed into PSUM col strips 0/32/64/96
                qpk = qp.tile([128, CHUNK * M], F32, tag="qps", name="qpk")
                for k in range(8):
                    lhsT = wq_sb[:, k * CP + 384:k * CP + 400]
                    for s in range(NCH):
                        rhs = xq_all[:, k * MT + s * CHUNK * M:
                                     k * MT + (s + 1) * CHUNK * M]
                        nc.tensor.matmul(qpk[32 * s:32 * s + 16, :], lhsT, rhs,
                                         start=(k == 0), stop=(k == 7),
                                         tile_position=(0, 32 * s))
                nc.scalar.activation(q3p[:, :], qpk[:, :], AF.Identity,
                                     bias=b4_sb[:, 0:1])
                fetch(5)

            with tc.tile_pool(name="ppsum", bufs=8, space="PSUM") as pp:
                for t in range(NT):
                    for s in range(NCH):
                        ps = pp.tile([128, CHUNK * M], F32, tag="pps", name="pps")
                        for kk in range(3):
                            lhsT = u_sb[:, kk * CP + t * 128:kk * CP + (t + 1) * 128]
                            rhs = q_sb[kk][:, s * CHUNK * M:(s + 1) * CHUNK * M]
                            nc.tensor.matmul(ps[:, :], lhsT, rhs,
                                             start=(kk == 0), stop=False)
                        # c4 contraction from the strip-packed q3p
                        nc.tensor.matmul(ps[:, :],
                                         u3_sb[32 * s:32 * s + 16, t * 128:(t + 1) * 128],
                                         q3p[32 * s:32 * s + 16, :],
                                         start=False, stop=True,
                                         tile_position=(32 * s, 0))
                        nc.vector.tensor_copy(
                            p_sb[t][:, s * CHUNK * M:(s + 1) * CHUNK * M], ps[:, :])

            _pa_cm.__exit__(None, None, None)

            # replicate P's c4 rows to strips 0/32/64/96 for the packed G kk=3
            for x in range(4):
                nc.sync.dma_start(p4r[32 * x:32 * x + 16, :], p_sb[3][0:16, :])

            # ---- Phase B: per-batch A encode + paired G + maxes + exp ----
            with tc.tile_pool(name="apsum", bufs=4, space="PSUM") as ap, \
                 tc.tile_pool(name="apk", bufs=2, space="PSUM") as apkp, \
                 tc.tile_pool(name="gpsum", bufs=2, space="PSUM") as gp, \
                 tc.tile_pool(name="ebc", bufs=2) as ebcp, \
                 tc.tile_pool(name="rows", bufs=4) as rowp, \
                 tc.tile_pool(name="tree", bufs=4) as trp:

                e_grp = {}
                apk = {}

                g_back_args = {}

                def g_front(bb, g2, off):
                    last = bb >= SPLIT
                    nc.vector.reduce_max(gq_all[0:40, bb:bb + 1], g2[off:off + 40, :],
                                         axis=mybir.AxisListType.X, op=OP.max)
                    e1q = rowp.tile([40, 1], F32, tag="e1q", name="e1q")
                    nc.scalar.activation(e1q[:, :], gq_all[0:40, bb:bb + 1],
                                         AF.Tanh)
                    nc.scalar.activation(e1q[:, :], e1q[:, :], AF.Exp)
                    nc.vector.tensor_add(sq_acc[1 if last else 0][:, :],
                                         sq_acc[1 if last else 0][:, :], e1q[:, :])
                    if last:
                        g_s = g_s4[:, (bb - SPLIT) * 400:(bb - SPLIT + 1) * 400]
                    else:
                        g_s = trp.tile([40, 400], F32, tag="g_s", name="g_s")
                    nc.scalar.activation(g_s[:, :], g2[off:off + 40, :], AF.Copy)
                    g_back_args[bb] = g_s

                def g_back(bb):
                    last = bb >= SPLIT
                    g_s = g_back_args.pop(bb)
                    g_r = trp.tile([40, 400], F32, tag="g_r", name="g_r")
                    nc.gpsimd.partition_all_reduce(
                        g_r[:, :], g_s[:, :], channels=40,
                        reduce_op=bass_isa.ReduceOp.max)
                    t1 = rowp.tile([1, 400], F32, tag="t1", name="t1")
                    nc.scalar.activation(t1[:, :], g_r[0:1, :], AF.Tanh)
                    e1 = rowp.tile([1, 400], F32, tag="e1", name="e1")
                    nc.scalar.activation(e1[:, :], t1[:, :], AF.Exp)
                    if not last:
                        nc.vector.tensor_add(s_acc[0][:, :], s_acc[0][:, :], e1[:, :])
                    e1b = rowp.tile([1, 400], BF16, tag="e1b", name="e1b")
                    nc.vector.tensor_copy(e1b[:, :], e1[:, :])
                    eg = e_grp[bb // GRP]
                    nc.gpsimd.partition_broadcast(
                        eg[:, (bb % GRP) * 400:(bb % GRP + 1) * 400], e1b[:, :])

                def g_chain(bb, g2, off):
                    g_front(bb, g2, off)
                    g_back(bb)

                g_pair_ps = {}

                def do_g_pair_mm(b0):
                    """G matmuls only; g2 psum kept for deferred fronts."""
                    g2 = gp.tile([128, 400], F32, tag="gps", name="gps")
                    for kk in range(3):
                        for j in range(2):
                            bb = b0 + j
                            nc.tensor.matmul(
                                g2[64 * j:64 * j + 40, :],
                                p_sb[kk][:, bb * M:(bb + 1) * M],
                                a_sb[kk][:, bb * L:(bb + 1) * L],
                                start=(kk == 0), stop=False,
                                tile_position=(0, 64 * j))
                    for j in range(2):
                        bb = b0 + j
                        x = bb % GRP
                        g = bb // GRP
                        nc.tensor.matmul(
                            g2[64 * j:64 * j + 40, :],
                            p4r[32 * x:32 * x + 16, bb * M:(bb + 1) * M],
                            a3p[32 * x:32 * x + 16, g * 400:(g + 1) * 400],
                            start=False, stop=(j == 1),
                            tile_position=(32 * x, 64 * j))
                    g_pair_ps[b0] = g2

                def do_g_pair(b0):
                    """G for batches b0, b0+1 packed via col groups 0/64."""
                    g2 = gp.tile([128, 400], F32, tag="gps", name="gps")
                    for kk in range(3):
                        for j in range(2):
                            bb = b0 + j
                            nc.tensor.matmul(
                                g2[64 * j:64 * j + 40, :],
                                p_sb[kk][:, bb * M:(bb + 1) * M],
                                a_sb[kk][:, bb * L:(bb + 1) * L],
                                start=(kk == 0), stop=False,
                                tile_position=(0, 64 * j))
                    for j in range(2):
                        bb = b0 + j
                        x = bb % GRP
                        g = bb // GRP
                        nc.tensor.matmul(
                            g2[64 * j:64 * j + 40, :],
                            p4r[32 * x:32 * x + 16, bb * M:(bb + 1) * M],
                            a3p[32 * x:32 * x + 16, g * 400:(g + 1) * 400],
                            start=False, stop=(j == 1),
                            tile_position=(32 * x, 64 * j))
                    g_chain(b0, g2, 0)
                    g_chain(b0 + 1, g2, 64)

                def y_mult(g):
                    # Y = A .* exp-broadcast over this group's slice
                    eg = e_grp.pop(g)
                    sl = slice(g * GRP * L, (g + 1) * GRP * L)
                    for t in range(3):
                        nc.vector.tensor_tensor(a_sb[t][:, sl], a_sb[t][:, sl],
                                                eg[:, :], op=OP.mult)
                    # strip-packed c4 tile: per-strip slices of eg line up
                    # with the per-batch strips of a3p
                    eg3 = ebcp.tile([128, 400], BF16, tag="eg3", name="eg3")
                    for x in range(GRP):
                        nc.vector.tensor_copy(
                            eg3[32 * x:32 * x + 16, :],
                            eg[32 * x:32 * x + 16, x * 400:(x + 1) * 400])
                    nc.vector.tensor_tensor(a3p[:, g * 400:(g + 1) * 400],
                                            a3p[:, g * 400:(g + 1) * 400],
                                            eg3[:, :], op=OP.mult)

                for bb in range(BS):
                    if bb + PRE < BS:
                        fetch(bb + PRE)
                    g = bb // GRP
                    x = bb % GRP
                    if x == 0:
                        e_grp[g] = ebcp.tile([128, GRP * 400], BF16,
                                             tag="e_g", name="e_g")
                        apk[g] = apkp.tile([128, 400], F32, tag="apk", name="apk")
                    xt = xts[bb]
                    for t in range(3):
                        aps = ap.tile([128, 400], F32, tag="aps", name="aps")
                        for k in range(8):
                            lhsT = wa_sb[:, k * CP + t * 128:k * CP + (t + 1) * 128]
                            nc.tensor.matmul(aps[:, :], lhsT, xt[:, k * 400:(k + 1) * 400],
                                             start=(k == 0), stop=(k == 7))
                        nc.scalar.activation(a_sb[t][:, bb * L:(bb + 1) * L],
                                             aps[:, :], AF.Identity,
                                             bias=ba_sb[:, t:t + 1])
                    if x == GRP - 1:
                        # c4 rows for the whole group: k-outer / strip-inner
                        # so the 4 col-group matmuls of each k overlap in
                        # the PE array
                        pk = apk.pop(g)
                        for k in range(8):
                            lhsT = wa_sb[:, k * CP + 384:k * CP + 400]
                            for x2 in range(GRP):
                                xt2 = xts[bb - 3 + x2]
                                nc.tensor.matmul(pk[32 * x2:32 * x2 + 16, :], lhsT,
                                                 xt2[:, k * 400:(k + 1) * 400],
                                                 start=(k == 0), stop=(k == 7),
                                                 tile_position=(0, 32 * x2))
                        nc.scalar.activation(a3p[:, g * 400:(g + 1) * 400],
                                             pk[:, :], AF.Identity,
                                             bias=b4_sb[:, 1:2])
                        for x2 in range(GRP):
                            xts.pop(bb - 3 + x2)
                        if bb < BS - 1:
                            do_g_pair(bb - 3)
                            do_g_pair(bb - 1)
                            y_mult(g)
                        else:
                            do_g_pair_mm(bb - 3)
                            do_g_pair_mm(bb - 1)
                            for b2 in range(SPLIT, BS):
                                g_front(b2, g_pair_ps[SPLIT + 2 * ((b2 - SPLIT) // 2)],
                                        64 * ((b2 - SPLIT) % 2))
                            # transposed AllGather-2 contribution on the now
                            # idle PE; its trigger goes ahead of the gpsimd
                            # PAR/broadcast tail
                            for q2 in range(4):
                                tpb = gp.tile([128, 400], F32, tag="gps", name="tpb")
                                for j in range(4):
                                    w = 128 if j < 3 else 16
                                    nc.tensor.transpose(
                                        tpb[0:w, 40 * j:40 * j + 40],
                                        g_s4[:, q2 * 400 + 128 * j:q2 * 400 + 128 * j + w],
                                        id_sb[0:40, 0:40])
                                nc.vector.reduce_max(
                                    et_all[:, 4 * q2:4 * q2 + 4],
                                    tpb[:, 0:160].rearrange("p (j m) -> p j m", j=4),
                                    axis=mybir.AxisListType.X, op=OP.max)
                            nc.scalar.activation(et_all[:, :], et_all[:, :], AF.Tanh)
                            nc.scalar.activation(et_all[:, :], et_all[:, :], AF.Exp)
                            et_v = et_all[:].rearrange("p (q j) -> p q j", q=4)
                            nc.vector.tensor_tensor(et_v[:, 0:2, :], et_v[:, 0:2, :],
                                                    et_v[:, 2:4, :], op=OP.add)
                            sT = rowp.tile([128, 4], F32, tag="sT", name="sT")
                            nc.vector.tensor_tensor(
                                sT[:, :].rearrange("p (o j) -> p o j", o=1),
                                et_v[:, 0:1, :], et_v[:, 1:2, :], op=OP.add)
                            nc.sync.dma_start(ccin[1][0:1, 0:40], sq_acc[1][:, :])
                            nc.sync.dma_start(
                                ccin[1][0:1, 40:424].rearrange("o (j i) -> i (o j)", j=3),
                                sT[:, 0:3])
                            nc.sync.dma_start(
                                ccin[1][0:1, 424:440].rearrange("o (j i) -> i (o j)", j=1),
                                sT[0:16, 3:4])
                            nc.gpsimd.collective_compute(
                                "AllGather", OP.bypass,
                                replica_groups=[list(range(N_CORES))],
                                ins=[ccin[1][:].opt()], outs=[ccout[1][:].opt()])
                            for b2 in range(SPLIT, BS):
                                g_back(b2)
                            y_mult(g)
                    if bb == SPLIT - 1:
                        nc.sync.dma_start(ccin[0][0:1, 0:40], sq_acc[0][:, :])
                        nc.sync.dma_start(ccin[0][0:1, 40:440], s_acc[0][:, :])
                        nc.gpsimd.collective_compute(
                            "AllGather", OP.bypass,
                            replica_groups=[list(range(N_CORES))],
                            ins=[ccin[0][:].opt()], outs=[ccout[0][:].opt()])

            _xap_cm.__exit__(None, None, None)
            _pab_cm.__exit__(None, None, None)

            # ---- Phase C: AllGather #2 + 1/S scale + pooled sums ----
            with tc.tile_pool(name="phc", bufs=1) as pc, \
                 tc.tile_pool(name="cpsum", bufs=2, space="PSUM") as cps, \
                 tc.tile_pool(name="cpsum1", bufs=2, space="PSUM") as cp1:
                sg = pc.tile([16, 440], F32, tag="sg", name="sg")
                nc.gpsimd.dma_start(sg[0:8, :], ccout[0][:, :])

                # --- work hidden under the collective: exp_q ---
                tq = pc.tile([40, BS], F32, tag="tq", name="tq")
                nc.scalar.activation(tq[:, :], gq_all[:, :], AF.Tanh)
                e_q = pc.tile([40, BS], F32, tag="e_q", name="e_q")
                nc.scalar.activation(e_q[:, :], tq[:, :], AF.Exp)
                eqt_ps = cp1.tile([BS, 40], F32, tag="c1", name="eqt_ps", bufs=1)
                nc.tensor.transpose(eqt_ps[:, :], e_q[:, :], id_sb[0:40, 0:40])
                eqt = pc.tile([BS, 40], BF16, tag="eqt", name="eqt")
                nc.vector.tensor_copy(eqt[:, :], eqt_ps[:, :])
                eq_fl = pc.tile([1, MT], BF16, tag="eq_fl", name="eq_fl")
                nc.gpsimd.dma_start(eq_fl[0:1, :], eqt[:, :])
                eq_bc = pc.tile([128, MT], BF16, tag="eq_bc", name="eq_bc")
                for ch in range(0, MT, 512):
                    ce = min(ch + 512, MT)
                    wqb = cps.tile([128, 512], F32, tag="wqb", name="wqb", bufs=1)
                    nc.tensor.matmul(wqb[:, 0:ce - ch], on_sb[:, :],
                                     eq_fl[0:1, ch:ce], start=True, stop=True)
                    nc.scalar.activation(eq_bc[:, ch:ce], wqb[:, 0:ce - ch], AF.Copy)
                for t in range(3):
                    nc.vector.tensor_tensor(q_sb[t][:, :], q_sb[t][:, :],
                                            eq_bc[:, :], op=OP.mult)
                eq3 = pc.tile([128, CHUNK * M], BF16, tag="eq3", name="eq3")
                for sx in range(NCH):
                    nc.vector.tensor_copy(
                        eq3[32 * sx:32 * sx + 16, :],
                        eq_bc[32 * sx:32 * sx + 16, sx * CHUNK * M:(sx + 1) * CHUNK * M])
                nc.vector.tensor_tensor(q3p[:, :], q3p[:, :], eq3[:, :], op=OP.mult)

                # --- post-collective: u = 1/S ---
                nc.gpsimd.dma_start(sg[8:16, :], ccout[1][:, :])
                sgp = cp1.tile([1, 440], F32, tag="sgp", name="sgp", bufs=1)
                nc.tensor.matmul(sgp[:, :], oc_sb[0:16, 0:1], sg[:, :],
                                 start=True, stop=True)
                ur = pc.tile([1, 440], F32, tag="ur", name="ur")
                nc.vector.reciprocal_approx_fast(ur[:, :], sgp[0:1, :])
                ub = pc.tile([1, 440], BF16, tag="ub", name="ub")
                nc.vector.tensor_copy(ub[:, :], ur[:, :])
                ua_bc = pc.tile([128, 400], BF16, tag="ua_bc", name="ua_bc")
                nc.gpsimd.partition_broadcast(ua_bc[:, :], ub[0:1, 40:440])

                uq_bc = pc.tile([128, 40], BF16, tag="uq_bc", name="uq_bc")
                nc.gpsimd.partition_broadcast(uq_bc[:, :], ub[0:1, 0:40])
                ua_v = ua_bc[:].rearrange("p (o l) -> p o l", o=1).broadcast_to((128, BS, 400))
                ua_v8 = ua_bc[:].rearrange("p (o l) -> p o l", o=1).broadcast_to((128, NG, 400))
                uq_v = uq_bc[:].rearrange("p (o m) -> p o m", o=1).broadcast_to((128, BS, 40))
                uq_v8 = uq_bc[:].rearrange("p (o m) -> p o m", o=1).broadcast_to((128, CHUNK, 40))

                def tree_sum(av, out, n, nb):
                    while n > 25 and n % 2 == 0:
                        h = n // 2
                        nc.vector.tensor_tensor(av[:, 0:nb, 0:h], av[:, 0:nb, 0:h],
                                                av[:, 0:nb, h:n], op=OP.add)
                        n = h
                    nc.vector.reduce_sum(out, av[:, 0:nb, 0:n],
                                         axis=mybir.AxisListType.X, op=OP.add)

                # c4 tile first: small, unblocks the repack DMA early
                a3v = a3p[:].rearrange("p (g l) -> p g l", g=NG)
                nc.vector.tensor_tensor(a3v, a3v, ua_v8, op=OP.mult)
                tree_sum(a3v, ra3p[:, :], 400, NG)
                # repack [strip x | c16, g] -> [c16, b=4g+x]
                for x in range(4):
                    nc.sync.dma_start(
                        ra_t[3][0:16, :].rearrange("i (g x) -> i x g", x=4)[:, x:x + 1, :],
                        ra3p[32 * x:32 * x + 16, :].rearrange("i (o g) -> i o g", o=1))
                scr = pc.tile([128, 400], BF16, tag="scr", name="scr")
                for t in range(3):
                    av = a_sb[t][:].rearrange("p (b l) -> p b l", b=BS)
                    nc.vector.tensor_tensor(av, av, ua_v, op=OP.mult)
                    if t == 0:
                        # otherwise-idle scalar engine reduces one c-tile
                        for bb in range(BS):
                            nc.scalar.activation(
                                scr[:, :], av[:, bb, :], AF.Copy,
                                accum_out=ra_t[t][:, bb:bb + 1])
                    elif t == 1:
                        for bb in range(12):
                            nc.scalar.activation(
                                scr[:, :], av[:, bb, :], AF.Copy,
                                accum_out=ra_t[t][:, bb:bb + 1])
                        tv = av[:, 12:BS, :]
                        n = 400
                        while n > 25 and n % 2 == 0:
                            h = n // 2
                            nc.vector.tensor_tensor(tv[:, :, 0:h], tv[:, :, 0:h],
                                                    tv[:, :, h:n], op=OP.add)
                            n = h
                        nc.vector.reduce_sum(ra_t[t][:, 12:BS], tv[:, :, 0:n],
                                             axis=mybir.AxisListType.X, op=OP.add)
                    else:
                        tree_sum(av, ra_t[t][:, :], 400, BS)
                q3v = q3p[:].rearrange("p (j m) -> p j m", j=CHUNK)
                nc.vector.tensor_tensor(q3v, q3v, uq_v8, op=OP.mult)
                tree_sum(q3v, rq3p[:, :], 40, CHUNK)
                for x in range(4):
                    nc.sync.dma_start(
                        rq_t[3][0:16, :].rearrange("i (s j) -> i s j", s=4)[:, x:x + 1, :],
                        rq3p[32 * x:32 * x + 16, :].rearrange("i (o j) -> i o j", o=1))

                for t in range(3):
                    qv = q_sb[t][:].rearrange("p (b m) -> p b m", b=BS)
                    nc.vector.tensor_tensor(qv, qv, uq_v, op=OP.mult)
                    tree_sum(qv, rq_t[t][:, :], 40, BS)

                # cosine similarity via accumulating PE transposes
                def psum_all(tiles, tag):
                    tps = cps.tile([BS, 128], F32, tag="cts", name=f"{tag}tp", bufs=2)
                    for t in range(NT):
                        nc.tensor.matmul(tps[:, :], tiles[t][:, :], id_sb[:, :],
                                         is_transpose=True,
                                         start=(t == 0), stop=(t == NT - 1))
                    col = pc.tile([32, 1], F32, tag=f"{tag}c", name=f"{tag}c")
                    nc.vector.reduce_sum(col[:, :], tps[:, :],
                                         axis=mybir.AxisListType.X, op=OP.add)
                    return col

                pr = [pc.tile([128, BS], F32, tag=f"pr{t}", name=f"pr{t}") for t in range(NT)]
                pq = [pc.tile([128, BS], F32, tag=f"pq{t}", name=f"pq{t}") for t in range(NT)]
                pa = [pc.tile([128, BS], F32, tag=f"pa{t}", name=f"pa{t}") for t in range(NT)]
                for t in range(NT):
                    nc.vector.tensor_mul(pr[t][:, :], rq_t[t][:, :], ra_t[t][:, :])
                    nc.vector.tensor_mul(pq[t][:, :], rq_t[t][:, :], rq_t[t][:, :])
                    nc.vector.tensor_mul(pa[t][:, :], ra_t[t][:, :], ra_t[t][:, :])
                dot = psum_all(pr, "dt")
                qq = psum_all(pq, "qq")
                aa = psum_all(pa, "aa")

                nq = pc.tile([32, 1], F32, tag="nq", name="nq")
                na = pc.tile([32, 1], F32, tag="na", name="na")
                nc.scalar.activation(nq[:, :], qq[:, :], AF.Sqrt)
                nc.scalar.activation(na[:, :], aa[:, :], AF.Sqrt)
                nc.vector.tensor_scalar_max(nq[:, :], nq[:, :], 1e-6)
                nc.vector.tensor_scalar_max(na[:, :], na[:, :], 1e-6)
                den = pc.tile([32, 1], F32, tag="den", name="den")
                nc.vector.tensor_mul(den[:, :], nq[:, :], na[:, :])
                rden = pc.tile([32, 1], F32, tag="rden", name="rden")
                nc.vector.reciprocal(rden[:, :], den[:, :])
                res = pc.tile([32, 1], F32, tag="res", name="res")
                nc.vector.tensor_mul(res[:, :], dot[:, :], rden[:, :])
                nc.gpsimd.dma_start(out_d[:].rearrange("(a b) -> a b", b=1),
                                    res[:, :])

    nc.finalize()
    return nc


def _prep(question, answer, Wq, bq, Wa, ba, U):
    bf = ml_dtypes.bfloat16
    qs = question.reshape(N_CORES, BS, M, E)
    as_ = answer.reshape(N_CORES, BS, L, E)

    def enc_z8(x, T):
        # x: [BS, T, E] -> Z^T rows [BS, 8, 128, T] bf16 (ctx shifts baked in)
        xt = x.transpose(0, 2, 1)  # [BS, E, T]
        xtp = np.zeros((x.shape[0], E, T + 2), np.float32)
        xtp[:, :, 1:T + 1] = xt
        z = np.zeros((x.shape[0], 1024, T), dtype=bf)
        for i in range(3):
            z[:, i * E:(i + 1) * E, :] = xtp[:, :, i:i + T].astype(bf)
        return z.reshape(x.shape[0], 8, 128, T)

    def enc_xq8(x):
        # [BS, M, E] -> [8, 128, BS*M] bf16
        z = enc_z8(x, M)  # [BS, 8, 128, 40]
        return np.ascontiguousarray(z.transpose(1, 2, 0, 3)).reshape(8, 128, MT)

    def enc_w8(W):
        # W [C, 900] -> W^T padded [8, 128, CP] bf16
        o = np.zeros((1024, CP), dtype=bf)
        o[0:900, 0:C] = W.T.astype(bf)
        return o.reshape(8, 128, CP)

    up = np.zeros((CP, CP), dtype=bf)
    up[0:C, 0:C] = U.astype(bf)
    up = up.reshape(NT, 128, CP)

    # U rows 384:400 replicated at partition strips 0/32/64/96
    u3r = np.zeros((128, CP), dtype=bf)
    for x in range(4):
        u3r[32 * x:32 * x + 16, 0:C] = U[384:400, :].astype(bf)

    def enc_b(b):
        o = np.zeros((CP,), np.float32)
        o[0:C] = b
        return np.ascontiguousarray(o.reshape(NT, 128).T)

    # bias rows 384:400 replicated at strips, for the packed c4 drains
    b4r = np.zeros((128, 2), np.float32)
    for x in range(4):
        b4r[32 * x:32 * x + 16, 0] = bq[384:400]
        b4r[32 * x:32 * x + 16, 1] = ba[384:400]

    com = {
        "wqt": enc_w8(Wq), "wat": enc_w8(Wa), "ut": up, "u3r": u3r,
        "bq": enc_b(bq), "ba": enc_b(ba), "b4r": b4r,
        "ident": np.eye(128, dtype=np.float32),
        "ones": np.ones((1, 128), dtype=bf),
        "ones_col": np.ones((128, 1), np.float32),
    }
    maps = []
    for i in range(N_CORES):
        m = dict(com)
        m["xq"] = enc_xq8(qs[i])
        m["xa"] = enc_z8(as_[i], L)
        maps.append(m)
    return maps


def kernel(question, answer, Wq, bq, Wa, ba, U, _trace=False):
    if "nc" not in _CACHE:
        _CACHE["nc"] = _build()
    nc = _CACHE["nc"]
    maps = _prep(np.asarray(question), np.asarray(answer), np.asarray(Wq),
                 np.asarray(bq), np.asarray(Wa), np.asarray(ba), np.asarray(U))
    r = run_bass_kernel_spmd(nc, maps, list(range(N_CORES)), trace=_trace)
    _CACHE["last"] = r
    return np.concatenate([r.results[i]["out"] for i in range(N_CORES)])


# revision 30
# speedup vs baseline: 1.1388x; 1.0178x over previous
"""AttentivePoolingNetwork Trainium2 kernel.

B=256 batch sharded 32/core across 8 NeuronCores. Per core:
  Q = cnn_encode(question)   [C=400, 32*40]   (bf16 matmuls, f32 psum)
  A = cnn_encode(answer)     [C=400, 32*400]  (kept in SBUF, bf16)
  P = U^T Q                  [C, 32*40]
  Gpre_b = P_b^T A_b         [40, 400] per batch item (tanh deferred)
  gq[b,m] = max_l Gpre, ga[b,l] = max_m Gpre (tanh applied after max)

C = 400 = 3*128 + 16: the 16-row 4th c-tile is packed 4-wide into PSUM
column strips (tile_position col groups) so its matmuls run concurrently
for 4 batches; G runs 2 batches per PSUM via col groups 0/64.

Softmax over the global batch dim via two AllGathers of the local
exp-sums (batches 0..27 early — latency hidden under phase B — then
28..31) + a PartitionAllReduce over the 16 gathered rows.
exp(tanh(ga)) rows are broadcast (GpSimd partition_broadcast) and
multiplied into A during phase B; post-collective work is the 1/S scale
and a free-dim reduce per c-tile split across DVE and ACT:
  rQ = Q w_q, rA = A w_a, out = cos(rQ, rA)
"""

import numpy as np
import ml_dtypes

import concourse.bass as bass
import concourse.tile as tile
from concourse import bacc, mybir
import concourse.bass_isa as bass_isa
from concourse.bass_utils import run_bass_kernel_spmd

F32 = mybir.dt.float32
BF16 = mybir.dt.bfloat16
AF = mybir.ActivationFunctionType
OP = mybir.AluOpType

N_CORES = 8
B, M, L, E, C = 256, 40, 400, 300, 400
BS = B // N_CORES          # 32 batch per core
CP = 512                   # C padded to 4*128
NT = CP // 128             # 4 c/d tiles
MT = BS * M                # 1280
LT = BS * L                # 12800
CHUNK = 8                  # batch chunk for Q/P matmuls (free dim 320)
NCH = BS // CHUNK          # 4
GRP = 4                    # batch group (e-broadcast + c4 strip packing)
NG = BS // GRP             # 8 groups
PRE = 6                    # xa prefetch depth (batches in flight)
SPLIT = 28                 # batches 0..SPLIT-1 go in the early AllGather

_CACHE = {}


def _build():
    nc = bacc.Bacc("TRN2", target_bir_lowering=False)

    xq_d = nc.dram_tensor("xq", [8, 128, MT], BF16, kind="ExternalInput")
    xa_d = nc.dram_tensor("xa", [BS, 8, 128, 400], BF16, kind="ExternalInput")
    wq_d = nc.dram_tensor("wqt", [8, 128, CP], BF16, kind="ExternalInput")
    wa_d = nc.dram_tensor("wat", [8, 128, CP], BF16, kind="ExternalInput")
    u_d = nc.dram_tensor("ut", [NT, 128, CP], BF16, kind="ExternalInput")
    u3_d = nc.dram_tensor("u3r", [128, CP], BF16, kind="ExternalInput")
    bq_d = nc.dram_tensor("bq", [128, NT], F32, kind="ExternalInput")
    ba_d = nc.dram_tensor("ba", [128, NT], F32, kind="ExternalInput")
    b4_d = nc.dram_tensor("b4r", [128, 2], F32, kind="ExternalInput")
    id_d = nc.dram_tensor("ident", [128, 128], F32, kind="ExternalInput")
    on_d = nc.dram_tensor("ones", [1, 128], BF16, kind="ExternalInput")
    oc_d = nc.dram_tensor("ones_col", [128, 1], F32, kind="ExternalInput")
    out_d = nc.dram_tensor("out", [32], F32, kind="ExternalOutput")

    with tile.TileContext(nc) as tc:
        with tc.tile_pool(name="const", bufs=1) as cp, \
             tc.tile_pool(name="dram", bufs=1, space="DRAM") as dp:
            # ---- persistent SBUF tensors ----
            wa_sb = cp.tile([128, 8 * CP], BF16, tag="wa_sb", name="wa_sb")
            bq_sb = cp.tile([128, NT], F32, tag="bq_sb", name="bq_sb")
            ba_sb = cp.tile([128, NT], F32, tag="ba_sb", name="ba_sb")
            b4_sb = cp.tile([128, 2], F32, tag="b4_sb", name="b4_sb")
            u3_sb = cp.tile([128, CP], BF16, tag="u3_sb", name="u3_sb")
            id_sb = cp.tile([128, 128], F32, tag="id_sb", name="id_sb")
            on_sb = cp.tile([1, 128], BF16, tag="on_sb", name="on_sb")
            oc_sb = cp.tile([128, 1], F32, tag="oc_sb", name="oc_sb")
            # last group's G rows (f32) + transposed-softmax accumulator
            g_s4 = cp.tile([40, 4 * 400], F32, tag="g_s4", name="g_s4")
            et_all = cp.tile([128, 16], F32, tag="et_all", name="et_all")
            # tiles 0..2 full 128 c-rows; tile 3 (c 384:400) strip-packed:
            # q3p strip s = Q4 of chunk s; a3p strip x col-group g = A4 of
            # batch 4g+x
            q_sb = [cp.tile([128, MT], BF16, tag=f"q_sb{t}", name=f"q_sb{t}") for t in range(3)]
            q3p = cp.tile([128, CHUNK * M], BF16, tag="q3p", name="q3p")
            a_sb = [cp.tile([128, LT], BF16, tag=f"a_sb{t}", name=f"a_sb{t}") for t in range(3)]
            a3p = cp.tile([128, NG * 400], BF16, tag="a3p", name="a3p")
            gq_all = cp.tile([40, BS], F32, tag="gq_all", name="gq_all")
            s_acc = [cp.tile([1, 400], F32, tag=f"s_acc{i}", name=f"s_acc{i}") for i in range(2)]
            sq_acc = [cp.tile([40, 1], F32, tag=f"sq_acc{i}", name=f"sq_acc{i}") for i in range(2)]
            rq_t = [cp.tile([128, BS], F32, tag=f"rq{t}", name=f"rq{t}") for t in range(NT)]
            ra_t = [cp.tile([128, BS], F32, tag=f"ra{t}", name=f"ra{t}") for t in range(NT)]
            rq3p = cp.tile([128, CHUNK], F32, tag="rq3p", name="rq3p")
            ra3p = cp.tile([128, NG], F32, tag="ra3p", name="ra3p")

            ccin = [dp.tile([1, 440], F32, tag=f"ccin{i}", name=f"ccin{i}") for i in range(2)]
            ccout = [dp.tile([8, 440], F32, tag=f"ccout{i}", name=f"ccout{i}") for i in range(2)]
            ccwi = dp.tile([1, 8], F32, tag="ccwi", name="ccwi")
            ccwo = dp.tile([1, 8], F32, tag="ccwo", name="ccwo")

            # ---- load constants/inputs ----
            _pab_cm = tc.tile_pool(name="pab", bufs=1)
            pab_pool = _pab_cm.__enter__()
            p_sb = [pab_pool.tile([128, MT], BF16, tag=f"p_sb{t}", name=f"p_sb{t}") for t in range(NT)]
            p4r = pab_pool.tile([128, MT], BF16, tag="p4r", name="p4r")
            _xap_cm = tc.tile_pool(name="xa_pool", bufs=PRE)
            xap = _xap_cm.__enter__()
            _pa_cm = tc.tile_pool(name="pa", bufs=1)
            pa_pool = _pa_cm.__enter__()
            xq_all = pa_pool.tile([128, 8 * MT], BF16, tag="xq_all", name="xq_all")
            wq_sb = pa_pool.tile([128, 8 * CP], BF16, tag="wq_sb", name="wq_sb")
            u_sb = pa_pool.tile([128, NT * CP], BF16, tag="u_sb", name="u_sb")
            # phase-A loads split over the sync and scalar queues so the
            # Q matmuls' inputs all land within the first ~12us
            for k in range(4):
                nc.sync.dma_start(wq_sb[:, k * CP:(k + 1) * CP], wq_d[k])
                nc.scalar.dma_start(wq_sb[:, (k + 4) * CP:(k + 5) * CP], wq_d[k + 4])
            for k in range(4):
                nc.sync.dma_start(xq_all[:, k * MT:k * MT + 640], xq_d[k][:, 0:640])
                nc.scalar.dma_start(xq_all[:, (k + 4) * MT:(k + 4) * MT + 640],
                                    xq_d[k + 4][:, 0:640])
            for k in range(4):
                nc.sync.dma_start(xq_all[:, k * MT + 640:(k + 1) * MT],
                                  xq_d[k][:, 640:MT])
                nc.scalar.dma_start(xq_all[:, (k + 4) * MT + 640:(k + 5) * MT],
                                    xq_d[k + 4][:, 640:MT])
            nc.sync.dma_start(bq_sb[:], bq_d[:])
            nc.scalar.dma_start(
                u_sb[:].rearrange("p (k d) -> p k d", k=NT),
                u_d[:].rearrange("k p d -> p k d"))
            nc.scalar.dma_start(
                wa_sb[:].rearrange("p (k c) -> p k c", k=8),
                wa_d[:].rearrange("k p c -> p k c"))
            nc.scalar.dma_start(ba_sb[:], ba_d[:])
            nc.scalar.dma_start(b4_sb[:], b4_d[:])
            nc.scalar.dma_start(u3_sb[:], u3_d[:])
            nc.sync.dma_start(id_sb[:], id_d[:])
            nc.sync.dma_start(on_sb[:], on_d[:])
            nc.sync.dma_start(oc_sb[:], oc_d[:])

            xts = {}

            def fetch(bb, eng=nc.gpsimd):
                xt = xap.tile([128, 8 * 400], BF16, tag="xa_t", name="xa_t")
                for h in range(2):
                    eng.dma_start(
                        xt[:, 1600 * h:1600 * (h + 1)].rearrange("p (k l) -> p k l", k=4),
                        xa_d[bb][4 * h:4 * (h + 1)].rearrange("k p l -> p k l"))
                xts[bb] = xt

            # stagger the prefetch so it doesn't crowd the phase-A loads
            fetch(0)
            fetch(1)

            # warm the ACT LUT table set (exp_and_others)
            warm = cp.tile([1, 32], F32, tag="warm", name="warm")
            nc.vector.memset(warm[:, :], 0.25)
            nc.scalar.activation(warm[:, :], warm[:, :], AF.Tanh)
            nc.scalar.activation(warm[:, :], warm[:, :], AF.Exp)

            # dummy collective: pre-pays the CC mesh wakeup
            wsync = cp.tile([1, 8], F32, tag="wsync", name="wsync")
            nc.vector.memset(wsync[:, :], 1.0)
            nc.gpsimd.dma_start(ccwi[0:1, :], wsync[:, :])
            nc.gpsimd.collective_compute(
                "AllReduce", OP.add,
                replica_groups=[list(range(N_CORES))],
                ins=[ccwi[:].opt()], outs=[ccwo[:].opt()])

            nc.vector.memset(s_acc[0][:, :], 0.0)
            nc.vector.memset(s_acc[1][:, :], 0.0)
            nc.vector.memset(sq_acc[0][:, :], 0.0)
            nc.vector.memset(sq_acc[1][:, :], 0.0)
            nc.vector.memset(rq_t[3][:, :], 0.0)
            nc.vector.memset(ra_t[3][:, :], 0.0)

            # ---- Phase A: Q encode + P = U^T Q ----
            with tc.tile_pool(name="qpsum", bufs=8, space="PSUM") as qp:
                for t in range(3):
                    ps = [qp.tile([128, CHUNK * M], F32, tag="qps", name="qps") for _ in range(NCH)]
                    for k in range(8):
                        lhsT = wq_sb[:, k * CP + t * 128:k * CP + (t + 1) * 128]
                        for s in range(NCH):
                            rhs = xq_all[:, k * MT + s * CHUNK * M:
                                         k * MT + (s + 1) * CHUNK * M]
                            nc.tensor.matmul(ps[s][:, :], lhsT, rhs,
                                             start=(k == 0), stop=(k == 7))
                    for s in range(NCH):
                        nc.vector.tensor_add(
                            q_sb[t][:, s * CHUNK * M:(s + 1) * CHUNK * M],
                            ps[s][:, :],
                            bq_sb[:, t:t + 1].broadcast_to((128, CHUNK * M)))
                    fetch(2 + t)
                # c4 tile: 4 chunks packed into PSUM col strips 0/32/64/96
                qpk = qp.tile([128, CHUNK * M], F32, tag="qps", name="qpk")
                for k in range(8):
                    lhsT = wq_sb[:, k * CP + 384:k * CP + 400]
                    for s in range(NCH):
                        rhs = xq_all[:, k * MT + s * CHUNK * M:
                                     k * MT + (s + 1) * CHUNK * M]
                        nc.tensor.matmul(qpk[32 * s:32 * s + 16, :], lhsT, rhs,
                                         start=(k == 0), stop=(k == 7),
                                         tile_position=(0, 32 * s))
                nc.scalar.activation(q3p[:, :], qpk[:, :], AF.Identity,
                                     bias=b4_sb[:, 0:1])
                fetch(5)

            with tc.tile_pool(name="ppsum", bufs=8, space="PSUM") as pp:
                for t in range(NT):
                    for s in range(NCH):
                        ps = pp.tile([128, CHUNK * M], F32, tag="pps", name="pps")
                        for kk in range(3):
                            lhsT = u_sb[:, kk * CP + t * 128:kk * CP + (t + 1) * 128]
                            rhs = q_sb[kk][:, s * CHUNK * M:(s + 1) * CHUNK * M]
                            nc.tensor.matmul(ps[:, :], lhsT, rhs,
                                             start=(kk == 0), stop=False)
                        # c4 contraction from the strip-packed q3p
                        nc.tensor.matmul(ps[:, :],
                                         u3_sb[32 * s:32 * s + 16, t * 128:(t + 1) * 128],
                                         q3p[32 * s:32 * s + 16, :],
                                         start=False, stop=True,
                                         tile_position=(32 * s, 0))
                        nc.vector.tensor_copy(
                            p_sb[t][:, s * CHUNK * M:(s + 1) * CHUNK * M], ps[:, :])

            _pa_cm.__exit__(None, None, None)

            # replicate P's c4 rows to strips 0/32/64/96 for the packed G kk=3
            for x in range(4):
                nc.sync.dma_start(p4r[32 * x:32 * x + 16, :], p_sb[3][0:16, :])

            # ---- Phase B: per-batch A encode + paired G + maxes + exp ----
            with tc.tile_pool(name="apsum", bufs=4, space="PSUM") as ap, \
                 tc.tile_pool(name="apk", bufs=2, space="PSUM") as apkp, \
                 tc.tile_pool(name="gpsum", bufs=2, space="PSUM") as gp, \
                 tc.tile_pool(name="ebc", bufs=2) as ebcp, \
                 tc.tile_pool(name="rows", bufs=4) as rowp, \
                 tc.tile_pool(name="tree", bufs=4) as trp:

                e_grp = {}
                apk = {}

                g_back_args = {}

                def g_front(bb, g2, off):
                    last = bb >= SPLIT
                    nc.vector.reduce_max(gq_all[0:40, bb:bb + 1], g2[off:off + 40, :],
                                         axis=mybir.AxisListType.X, op=OP.max)
                    e1q = rowp.tile([40, 1], F32, tag="e1q", name="e1q")
                    nc.scalar.activation(e1q[:, :], gq_all[0:40, bb:bb + 1],
                                         AF.Tanh)
                    nc.scalar.activation(e1q[:, :], e1q[:, :], AF.Exp)
                    nc.vector.tensor_add(sq_acc[1 if last else 0][:, :],
                                         sq_acc[1 if last else 0][:, :], e1q[:, :])
                    if last:
                        g_s = g_s4[:, (bb - SPLIT) * 400:(bb - SPLIT + 1) * 400]
                    else:
                        g_s = trp.tile([40, 400], F32, tag="g_s", name="g_s")
                    nc.scalar.activation(g_s[:, :], g2[off:off + 40, :], AF.Copy)
                    g_back_args[bb] = g_s

                def g_back(bb):
                    last = bb >= SPLIT
                    g_s = g_back_args.pop(bb)
                    g_r = trp.tile([40, 400], F32, tag="g_r", name="g_r")
                    nc.gpsimd.partition_all_reduce(
                        g_r[:, :], g_s[:, :], channels=40,
                        reduce_op=bass_isa.ReduceOp.max)
                    t1 = rowp.tile([1, 400], F32, tag="t1", name="t1")
                    nc.scalar.activation(t1[:, :], g_r[0:1, :], AF.Tanh)
                    e1 = rowp.tile([1, 400], F32, tag="e1", name="e1")
                    nc.scalar.activation(e1[:, :], t1[:, :], AF.Exp)
                    if not last:
                        nc.vector.tensor_add(s_acc[0][:, :], s_acc[0][:, :], e1[:, :])
                    e1b = rowp.tile([1, 400], BF16, tag="e1b", name="e1b")
                    nc.vector.tensor_copy(e1b[:, :], e1[:, :])
                    eg = e_grp[bb // GRP]
                    nc.gpsimd.partition_broadcast(
                        eg[:, (bb % GRP) * 400:(bb % GRP + 1) * 400], e1b[:, :])

                def g_chain(bb, g2, off):
                    g_front(bb, g2, off)
                    g_back(bb)

                g_pair_ps = {}

                def do_g_pair_mm(b0):
                    """G matmuls only; g2 psum kept for deferred fronts."""
                    g2 = gp.tile([128, 400], F32, tag="gps", name="gps")
                    for kk in range(3):
                        for j in range(2):
                            bb = b0 + j
                            nc.tensor.matmul(
                                g2[64 * j:64 * j + 40, :],
                                p_sb[kk][:, bb * M:(bb + 1) * M],
                                a_sb[kk][:, bb * L:(bb + 1) * L],
                                start=(kk == 0), stop=False,
                                tile_position=(0, 64 * j))
                    for j in range(2):
                        bb = b0 + j
                        x = bb % GRP
                        g = bb // GRP
                        nc.tensor.matmul(
                            g2[64 * j:64 * j + 40, :],
                            p4r[32 * x:32 * x + 16, bb * M:(bb + 1) * M],
                            a3p[32 * x:32 * x + 16, g * 400:(g + 1) * 400],
                            start=False, stop=(j == 1),
                            tile_position=(32 * x, 64 * j))
                    g_pair_ps[b0] = g2

                def do_g_pair(b0):
                    """G for batches b0, b0+1 packed via col groups 0/64."""
                    g2 = gp.tile([128, 400], F32, tag="gps", name="gps")
                    for kk in range(3):
                        for j in range(2):
                            bb = b0 + j
                            nc.tensor.matmul(
                                g2[64 * j:64 * j + 40, :],
                                p_sb[kk][:, bb * M:(bb + 1) * M],
                                a_sb[kk][:, bb * L:(bb + 1) * L],
                                start=(kk == 0), stop=False,
                                tile_position=(0, 64 * j))
                    for j in range(2):
                        bb = b0 + j
                        x = bb % GRP
                        g = bb // GRP
                        nc.tensor.matmul(
                            g2[64 * j:64 * j + 40, :],
                            p4r[32 * x:32 * x + 16, bb * M:(bb + 1) * M],
                            a3p[32 * x:32 * x + 16, g * 400:(g + 1) * 400],
                            start=False, stop=(j == 1),
                            tile_position=(32 * x, 64 * j))
                    g_chain(b0, g2, 0)
                    g_chain(b0 + 1, g2, 64)

                def y_mult(g):
                    # Y = A .* exp-broadcast over this group's slice
                    eg = e_grp.pop(g)
                    sl = slice(g * GRP * L, (g + 1) * GRP * L)
                    for t in range(3):
                        nc.vector.tensor_tensor(a_sb[t][:, sl], a_sb[t][:, sl],
                                                eg[:, :], op=OP.mult)
                    # strip-packed c4 tile: per-strip slices of eg line up
                    # with the per-batch strips of a3p
                    eg3 = ebcp.tile([128, 400], BF16, tag="eg3", name="eg3")
                    for x in range(GRP):
                        nc.vector.tensor_copy(
                            eg3[32 * x:32 * x + 16, :],
                            eg[32 * x:32 * x + 16, x * 400:(x + 1) * 400])
                    nc.vector.tensor_tensor(a3p[:, g * 400:(g + 1) * 400],
                                            a3p[:, g * 400:(g + 1) * 400],
                                            eg3[:, :], op=OP.mult)

                for bb in range(BS):
                    if bb + PRE < BS:
                        fetch(bb + PRE)
                    g = bb // GRP
                    x = bb % GRP
                    if x == 0:
                        e_grp[g] = ebcp.tile([128, GRP * 400], BF16,
                                             tag="e_g", name="e_g")
                        apk[g] = apkp.tile([128, 400], F32, tag="apk", name="apk")
                    xt = xts[bb]
                    for t in range(3):
                        aps = ap.tile([128, 400], F32, tag="aps", name="aps")
                        for k in range(8):
                            lhsT = wa_sb[:, k * CP + t * 128:k * CP + (t + 1) * 128]
                            nc.tensor.matmul(aps[:, :], lhsT, xt[:, k * 400:(k + 1) * 400],
                                             start=(k == 0), stop=(k == 7))
                        nc.scalar.activation(a_sb[t][:, bb * L:(bb + 1) * L],
                                             aps[:, :], AF.Identity,
                                             bias=ba_sb[:, t:t + 1])
                    if x == GRP - 1:
                        # c4 rows for the whole group: k-outer / strip-inner
                        # so the 4 col-group matmuls of each k overlap in
                        # the PE array
                        pk = apk.pop(g)
                        for k in range(8):
                            lhsT = wa_sb[:, k * CP + 384:k * CP + 400]
                            for x2 in range(GRP):
                                xt2 = xts[bb - 3 + x2]
                                nc.tensor.matmul(pk[32 * x2:32 * x2 + 16, :], lhsT,
                                                 xt2[:, k * 400:(k + 1) * 400],
                                                 start=(k == 0), stop=(k == 7),
                                                 tile_position=(0, 32 * x2))
                        nc.scalar.activation(a3p[:, g * 400:(g + 1) * 400],
                                             pk[:, :], AF.Identity,
                                             bias=b4_sb[:, 1:2])
                        for x2 in range(GRP):
                            xts.pop(bb - 3 + x2)
                        if bb < BS - 1:
                            do_g_pair(bb - 3)
                            do_g_pair(bb - 1)
                            y_mult(g)
                        else:
                            do_g_pair_mm(bb - 3)
                            do_g_pair_mm(bb - 1)
                            for b2 in range(SPLIT, BS):
                                g_front(b2, g_pair_ps[SPLIT + 2 * ((b2 - SPLIT) // 2)],
                                        64 * ((b2 - SPLIT) % 2))
                            # transposed AllGather-2 contribution on the now
                            # idle PE; its trigger goes ahead of the gpsimd
                            # PAR/broadcast tail
                            for q2 in range(4):
                                tpb = gp.tile([128, 400], F32, tag="gps", name="tpb")
                                for j in range(4):
                                    w = 128 if j < 3 else 16
                                    nc.tensor.transpose(
                                        tpb[0:w, 40 * j:40 * j + 40],
                                        g_s4[:, q2 * 400 + 128 * j:q2 * 400 + 128 * j + w],
                                        id_sb[0:40, 0:40])
                                nc.vector.reduce_max(
                                    et_all[:, 4 * q2:4 * q2 + 4],
                                    tpb[:, 0:160].rearrange("p (j m) -> p j m", j=4),
                                    axis=mybir.AxisListType.X, op=OP.max)
                            nc.scalar.activation(et_all[:, :], et_all[:, :], AF.Tanh)
                            nc.scalar.activation(et_all[:, :], et_all[:, :], AF.Exp)
                            et_v = et_all[:].rearrange("p (q j) -> p q j", q=4)
                            nc.vector.tensor_tensor(et_v[:, 0:2, :], et_v[:, 0:2, :],
                                                    et_v[:, 2:4, :], op=OP.add)
                            sT = rowp.tile([128, 4], F32, tag="sT", name="sT")
                            nc.vector.tensor_tensor(
                                sT[:, :].rearrange("p (o j) -> p o j", o=1),
                                et_v[:, 0:1, :], et_v[:, 1:2, :], op=OP.add)
                            nc.sync.dma_start(ccin[1][0:1, 0:40], sq_acc[1][:, :])
                            nc.sync.dma_start(
                                ccin[1][0:1, 40:424].rearrange("o (j i) -> i (o j)", j=3),
                                sT[:, 0:3])
                            nc.sync.dma_start(
                                ccin[1][0:1, 424:440].rearrange("o (j i) -> i (o j)", j=1),
                                sT[0:16, 3:4])
                            nc.gpsimd.collective_compute(
                                "AllGather", OP.bypass,
                                replica_groups=[list(range(N_CORES))],
                                ins=[ccin[1][:].opt()], outs=[ccout[1][:].opt()])
                            for b2 in range(SPLIT, BS):
                                g_back(b2)
                            y_mult(g)
                    if bb == SPLIT - 1:
                        nc.sync.dma_start(ccin[0][0:1, 0:40], sq_acc[0][:, :])
                        nc.sync.dma_start(ccin[0][0:1, 40:440], s_acc[0][:, :])
                        nc.gpsimd.collective_compute(
                            "AllGather", OP.bypass,
                            replica_groups=[list(range(N_CORES))],
                            ins=[ccin[0][:].opt()], outs=[ccout[0][:].opt()])

            _xap_cm.__exit__(None, None, None)
            _pab_cm.__exit__(None, None, None)

            # ---- Phase C: AllGather #2 + 1/S scale + pooled sums ----
            with tc.tile_pool(name="phc", bufs=1) as pc, \
                 tc.tile_pool(name="cpsum", bufs=2, space="PSUM") as cps, \
                 tc.tile_pool(name="cpsum1", bufs=2, space="PSUM") as cp1:
                sg = pc.tile([16, 440], F32, tag="sg", name="sg")
                nc.scalar.dma_start(sg[0:8, :], ccout[0][:, :])

                # --- work hidden under the collective: exp_q ---
                tq = pc.tile([40, BS], F32, tag="tq", name="tq")
                nc.scalar.activation(tq[:, :], gq_all[:, :], AF.Tanh)
                e_q = pc.tile([40, BS], F32, tag="e_q", name="e_q")
                nc.scalar.activation(e_q[:, :], tq[:, :], AF.Exp)
                eqt_ps = cp1.tile([BS, 40], F32, tag="c1", name="eqt_ps", bufs=1)
                nc.tensor.transpose(eqt_ps[:, :], e_q[:, :], id_sb[0:40, 0:40])
                eqt = pc.tile([BS, 40], BF16, tag="eqt", name="eqt")
                nc.vector.tensor_copy(eqt[:, :], eqt_ps[:, :])
                eq_fl = pc.tile([1, MT], BF16, tag="eq_fl", name="eq_fl")
                nc.gpsimd.dma_start(eq_fl[0:1, :], eqt[:, :])
                eq_bc = pc.tile([128, MT], BF16, tag="eq_bc", name="eq_bc")
                for ch in range(0, MT, 512):
                    ce = min(ch + 512, MT)
                    wqb = cps.tile([128, 512], F32, tag="wqb", name="wqb", bufs=1)
                    nc.tensor.matmul(wqb[:, 0:ce - ch], on_sb[:, :],
                                     eq_fl[0:1, ch:ce], start=True, stop=True)
                    nc.scalar.activation(eq_bc[:, ch:ce], wqb[:, 0:ce - ch], AF.Copy)
                for t in range(3):
                    nc.vector.tensor_tensor(q_sb[t][:, :], q_sb[t][:, :],
                                            eq_bc[:, :], op=OP.mult)
                eq3 = pc.tile([128, CHUNK * M], BF16, tag="eq3", name="eq3")
                for sx in range(NCH):
                    nc.vector.tensor_copy(
                        eq3[32 * sx:32 * sx + 16, :],
                        eq_bc[32 * sx:32 * sx + 16, sx * CHUNK * M:(sx + 1) * CHUNK * M])
                nc.vector.tensor_tensor(q3p[:, :], q3p[:, :], eq3[:, :], op=OP.mult)

                # --- post-collective: u = 1/S ---
                nc.scalar.dma_start(sg[8:16, :], ccout[1][:, :])
                sgp = cp1.tile([1, 440], F32, tag="sgp", name="sgp", bufs=1)
                nc.tensor.matmul(sgp[:, :], oc_sb[0:16, 0:1], sg[:, :],
                                 start=True, stop=True)
                ur = pc.tile([1, 440], F32, tag="ur", name="ur")
                nc.vector.reciprocal_approx_fast(ur[:, :], sgp[0:1, :])
                ub = pc.tile([1, 440], BF16, tag="ub", name="ub")
                nc.vector.tensor_copy(ub[:, :], ur[:, :])
                ua_bc = pc.tile([128, 400], BF16, tag="ua_bc", name="ua_bc")
                nc.gpsimd.partition_broadcast(ua_bc[:, :], ub[0:1, 40:440])

                uq_bc = pc.tile([128, 40], BF16, tag="uq_bc", name="uq_bc")
                nc.gpsimd.partition_broadcast(uq_bc[:, :], ub[0:1, 0:40])
                ua_v = ua_bc[:].rearrange("p (o l) -> p o l", o=1).broadcast_to((128, BS, 400))
                ua_v8 = ua_bc[:].rearrange("p (o l) -> p o l", o=1).broadcast_to((128, NG, 400))
                uq_v = uq_bc[:].rearrange("p (o m) -> p o m", o=1).broadcast_to((128, BS, 40))
                uq_v8 = uq_bc[:].rearrange("p (o m) -> p o m", o=1).broadcast_to((128, CHUNK, 40))

                def tree_sum(av, out, n, nb):
                    while n > 25 and n % 2 == 0:
                        h = n // 2
                        nc.vector.tensor_tensor(av[:, 0:nb, 0:h], av[:, 0:nb, 0:h],
                                                av[:, 0:nb, h:n], op=OP.add)
                        n = h
                    nc.vector.reduce_sum(out, av[:, 0:nb, 0:n],
                                         axis=mybir.AxisListType.X, op=OP.add)

                # c4 tile first: small, unblocks the repack DMA early
                a3v = a3p[:].rearrange("p (g l) -> p g l", g=NG)
                nc.vector.tensor_tensor(a3v, a3v, ua_v8, op=OP.mult)
                tree_sum(a3v, ra3p[:, :], 400, NG)
                # repack [strip x | c16, g] -> [c16, b=4g+x]
                for x in range(4):
                    nc.sync.dma_start(
                        ra_t[3][0:16, :].rearrange("i (g x) -> i x g", x=4)[:, x:x + 1, :],
                        ra3p[32 * x:32 * x + 16, :].rearrange("i (o g) -> i o g", o=1))
                scr = pc.tile([128, 400], BF16, tag="scr", name="scr")
                for t in range(3):
                    av = a_sb[t][:].rearrange("p (b l) -> p b l", b=BS)
                    nc.vector.tensor_tensor(av, av, ua_v, op=OP.mult)
                    if t == 0:
                        # otherwise-idle scalar engine reduces one c-tile
                        for bb in range(BS):
                            nc.scalar.activation(
                                scr[:, :], av[:, bb, :], AF.Copy,
                                accum_out=ra_t[t][:, bb:bb + 1])
                    elif t == 1:
                        for bb in range(12):
                            nc.scalar.activation(
                                scr[:, :], av[:, bb, :], AF.Copy,
                                accum_out=ra_t[t][:, bb:bb + 1])
                        tv = av[:, 12:BS, :]
                        n = 400
                        while n > 25 and n % 2 == 0:
                            h = n // 2
                            nc.vector.tensor_tensor(tv[:, :, 0:h], tv[:, :, 0:h],
                                                    tv[:, :, h:n], op=OP.add)
                            n = h
                        nc.vector.reduce_sum(ra_t[t][:, 12:BS], tv[:, :, 0:n],
                                             axis=mybir.AxisListType.X, op=OP.add)
                    else:
                        tree_sum(av, ra_t[t][:, :], 400, BS)
                q3v = q3p[:].rearrange("p (j m) -> p j m", j=CHUNK)
                nc.vector.tensor_tensor(q3v, q3v, uq_v8, op=OP.mult)
                tree_sum(q3v, rq3p[:, :], 40, CHUNK)
                for x in range(4):
                    nc.sync.dma_start(
                        rq_t[3][0:16, :].rearrange("i (s j) -> i s j", s=4)[:, x:x + 1, :],
                        rq3p[32 * x:32 * x + 16, :].rearrange("i (o j) -> i o j", o=1))

                for t in range(3):
                    qv = q_sb[t][:].rearrange("p (b m) -> p b m", b=BS)
                    nc.vector.tensor_tensor(qv, qv, uq_v, op=OP.mult)
                    tree_sum(qv, rq_t[t][:, :], 40, BS)

                # cosine similarity via accumulating PE transposes
                def psum_all(tiles, tag):
                    tps = cps.tile([BS, 128], F32, tag="cts", name=f"{tag}tp", bufs=2)
                    for t in range(NT):
                        nc.tensor.matmul(tps[:, :], tiles[t][:, :], id_sb[:, :],
                                         is_transpose=True,
                                         start=(t == 0), stop=(t == NT - 1))
                    col = pc.tile([32, 1], F32, tag=f"{tag}c", name=f"{tag}c")
                    nc.vector.reduce_sum(col[:, :], tps[:, :],
                                         axis=mybir.AxisListType.X, op=OP.add)
                    return col

                pr = [pc.tile([128, BS], F32, tag=f"pr{t}", name=f"pr{t}") for t in range(NT)]
                pq = [pc.tile([128, BS], F32, tag=f"pq{t}", name=f"pq{t}") for t in range(NT)]
                pa = [pc.tile([128, BS], F32, tag=f"pa{t}", name=f"pa{t}") for t in range(NT)]
                for t in range(NT):
                    nc.vector.tensor_mul(pr[t][:, :], rq_t[t][:, :], ra_t[t][:, :])
                    nc.vector.tensor_mul(pq[t][:, :], rq_t[t][:, :], rq_t[t][:, :])
                    nc.vector.tensor_mul(pa[t][:, :], ra_t[t][:, :], ra_t[t][:, :])
                dot = psum_all(pr, "dt")
                qq = psum_all(pq, "qq")
                aa = psum_all(pa, "aa")

                nq = pc.tile([32, 1], F32, tag="nq", name="nq")
                na = pc.tile([32, 1], F32, tag="na", name="na")
                nc.scalar.activation(nq[:, :], qq[:, :], AF.Sqrt)
                nc.scalar.activation(na[:, :], aa[:, :], AF.Sqrt)
                nc.vector.tensor_scalar_max(nq[:, :], nq[:, :], 1e-6)
                nc.vector.tensor_scalar_max(na[:, :], na[:, :], 1e-6)
                den = pc.tile([32, 1], F32, tag="den", name="den")
                nc.vector.tensor_mul(den[:, :], nq[:, :], na[:, :])
                rden = pc.tile([32, 1], F32, tag="rden", name="rden")
                nc.vector.reciprocal(rden[:, :], den[:, :])
                res = pc.tile([32, 1], F32, tag="res", name="res")
                nc.vector.tensor_mul(res[:, :], dot[:, :], rden[:, :])
                nc.gpsimd.dma_start(out_d[:].rearrange("(a b) -> a b", b=1),
                                    res[:, :])

    nc.finalize()
    return nc


def _prep(question, answer, Wq, bq, Wa, ba, U):
    bf = ml_dtypes.bfloat16
    qs = question.reshape(N_CORES, BS, M, E)
    as_ = answer.reshape(N_CORES, BS, L, E)

    def enc_z8(x, T):
        # x: [BS, T, E] -> Z^T rows [BS, 8, 128, T] bf16 (ctx shifts baked in)
        xt = x.transpose(0, 2, 1)  # [BS, E, T]
        xtp = np.zeros((x.shape[0], E, T + 2), np.float32)
        xtp[:, :, 1:T + 1] = xt
        z = np.zeros((x.shape[0], 1024, T), dtype=bf)
        for i in range(3):
            z[:, i * E:(i + 1) * E, :] = xtp[:, :, i:i + T].astype(bf)
        return z.reshape(x.shape[0], 8, 128, T)

    def enc_xq8(x):
        # [BS, M, E] -> [8, 128, BS*M] bf16
        z = enc_z8(x, M)  # [BS, 8, 128, 40]
        return np.ascontiguousarray(z.transpose(1, 2, 0, 3)).reshape(8, 128, MT)

    def enc_w8(W):
        # W [C, 900] -> W^T padded [8, 128, CP] bf16
        o = np.zeros((1024, CP), dtype=bf)
        o[0:900, 0:C] = W.T.astype(bf)
        return o.reshape(8, 128, CP)

    up = np.zeros((CP, CP), dtype=bf)
    up[0:C, 0:C] = U.astype(bf)
    up = up.reshape(NT, 128, CP)

    # U rows 384:400 replicated at partition strips 0/32/64/96
    u3r = np.zeros((128, CP), dtype=bf)
    for x in range(4):
        u3r[32 * x:32 * x + 16, 0:C] = U[384:400, :].astype(bf)

    def enc_b(b):
        o = np.zeros((CP,), np.float32)
        o[0:C] = b
        return np.ascontiguousarray(o.reshape(NT, 128).T)

    # bias rows 384:400 replicated at strips, for the packed c4 drains
    b4r = np.zeros((128, 2), np.float32)
    for x in range(4):
        b4r[32 * x:32 * x + 16, 0] = bq[384:400]
        b4r[32 * x:32 * x + 16, 1] = ba[384:400]

    com = {
        "wqt": enc_w8(Wq), "wat": enc_w8(Wa), "ut": up, "u3r": u3r,
        "bq": enc_b(bq), "ba": enc_b(ba), "b4r": b4r,
        "ident": np.eye(128, dtype=np.float32),
        "ones": np.ones((1, 128), dtype=bf),
        "ones_col": np.ones((128, 1), np.float32),
    }
    maps = []
    for i in range(N_CORES):
        m = dict(com)
        m["xq"] = enc_xq8(qs[i])
        m["xa"] = enc_z8(as_[i], L)
        maps.append(m)
    return maps


def kernel(question, answer, Wq, bq, Wa, ba, U, _trace=False):
    if "nc" not in _CACHE:
        _CACHE["nc"] = _build()
    nc = _CACHE["nc"]
    maps = _prep(np.asarray(question), np.asarray(answer), np.asarray(Wq),
                 np.asarray(bq), np.asarray(Wa), np.asarray(ba), np.asarray(U))
    r = run_bass_kernel_spmd(nc, maps, list(range(N_CORES)), trace=_trace)
    _CACHE["last"] = r
    return np.concatenate([r.results[i]["out"] for i in range(N_CORES)])


# revision 31
# speedup vs baseline: 1.2001x; 1.0538x over previous
"""AttentivePoolingNetwork Trainium2 kernel.

B=256 batch sharded 32/core across 8 NeuronCores. Per core:
  Q = cnn_encode(question)   [C=400, 32*40]   (bf16 matmuls, f32 psum)
  A = cnn_encode(answer)     [C=400, 32*400]  (kept in SBUF, bf16)
  P = U^T Q                  [C, 32*40]
  Gpre_b = P_b^T A_b         [40, 400] per batch item (tanh deferred)
  gq[b,m] = max_l Gpre, ga[b,l] = max_m Gpre (tanh applied after max)

C = 400 = 3*128 + 16: the 16-row 4th c-tile is packed 4-wide into PSUM
column strips (tile_position col groups) so its matmuls run concurrently
for 4 batches; G runs 2 batches per PSUM via col groups 0/64.

Softmax over the global batch dim via two AllGathers of the local
exp-sums (batches 0..27 early — latency hidden under phase B — then
28..31) + a PartitionAllReduce over the 16 gathered rows.
exp(tanh(ga)) rows are broadcast (GpSimd partition_broadcast) and
multiplied into A during phase B; post-collective work is the 1/S scale
and a free-dim reduce per c-tile split across DVE and ACT:
  rQ = Q w_q, rA = A w_a, out = cos(rQ, rA)
"""

import numpy as np
import ml_dtypes

import concourse.bass as bass
import concourse.tile as tile
from concourse import bacc, mybir
import concourse.bass_isa as bass_isa
from concourse.bass_utils import run_bass_kernel_spmd

F32 = mybir.dt.float32
BF16 = mybir.dt.bfloat16
AF = mybir.ActivationFunctionType
OP = mybir.AluOpType

N_CORES = 8
B, M, L, E, C = 256, 40, 400, 300, 400
BS = B // N_CORES          # 32 batch per core
CP = 512                   # C padded to 4*128
NT = CP // 128             # 4 c/d tiles
MT = BS * M                # 1280
LT = BS * L                # 12800
CHUNK = 8                  # batch chunk for Q/P matmuls (free dim 320)
NCH = BS // CHUNK          # 4
GRP = 4                    # batch group (e-broadcast + c4 strip packing)
NG = BS // GRP             # 8 groups
PRE = 6                    # xa prefetch depth (batches in flight)
SPLIT = 28                 # batches 0..SPLIT-1 go in the early AllGather

_CACHE = {}


def _build():
    nc = bacc.Bacc("TRN2", target_bir_lowering=False)

    xq_d = nc.dram_tensor("xq", [8, 128, MT], BF16, kind="ExternalInput")
    xa_d = nc.dram_tensor("xa", [BS, 8, 128, 400], BF16, kind="ExternalInput")
    wq_d = nc.dram_tensor("wqt", [8, 128, CP], BF16, kind="ExternalInput")
    wa_d = nc.dram_tensor("wat", [8, 128, CP], BF16, kind="ExternalInput")
    u_d = nc.dram_tensor("ut", [NT, 128, CP], BF16, kind="ExternalInput")
    u3_d = nc.dram_tensor("u3r", [128, CP], BF16, kind="ExternalInput")
    bq_d = nc.dram_tensor("bq", [128, NT], F32, kind="ExternalInput")
    ba_d = nc.dram_tensor("ba", [128, NT], F32, kind="ExternalInput")
    b4_d = nc.dram_tensor("b4r", [128, 2], F32, kind="ExternalInput")
    id_d = nc.dram_tensor("ident", [128, 128], F32, kind="ExternalInput")
    on_d = nc.dram_tensor("ones", [1, 128], BF16, kind="ExternalInput")
    oc_d = nc.dram_tensor("ones_col", [128, 1], F32, kind="ExternalInput")
    out_d = nc.dram_tensor("out", [32], F32, kind="ExternalOutput")

    with tile.TileContext(nc) as tc:
        with tc.tile_pool(name="const", bufs=1) as cp, \
             tc.tile_pool(name="dram", bufs=1, space="DRAM") as dp:
            # ---- persistent SBUF tensors ----
            wa_sb = cp.tile([128, 8 * CP], BF16, tag="wa_sb", name="wa_sb")
            bq_sb = cp.tile([128, NT], F32, tag="bq_sb", name="bq_sb")
            ba_sb = cp.tile([128, NT], F32, tag="ba_sb", name="ba_sb")
            b4_sb = cp.tile([128, 2], F32, tag="b4_sb", name="b4_sb")
            u3_sb = cp.tile([128, CP], BF16, tag="u3_sb", name="u3_sb")
            id_sb = cp.tile([128, 128], F32, tag="id_sb", name="id_sb")
            on_sb = cp.tile([1, 128], BF16, tag="on_sb", name="on_sb")
            oc_sb = cp.tile([128, 1], F32, tag="oc_sb", name="oc_sb")
            # last group's G rows (f32) + transposed-softmax accumulator
            g_s4 = cp.tile([40, 4 * 400], F32, tag="g_s4", name="g_s4")
            et_all = cp.tile([128, 16], F32, tag="et_all", name="et_all")
            # tiles 0..2 full 128 c-rows; tile 3 (c 384:400) strip-packed:
            # q3p strip s = Q4 of chunk s; a3p strip x col-group g = A4 of
            # batch 4g+x
            q_sb = [cp.tile([128, MT], BF16, tag=f"q_sb{t}", name=f"q_sb{t}") for t in range(3)]
            q3p = cp.tile([128, CHUNK * M], BF16, tag="q3p", name="q3p")
            a_sb = [cp.tile([128, LT], BF16, tag=f"a_sb{t}", name=f"a_sb{t}") for t in range(3)]
            a3p = cp.tile([128, NG * 400], BF16, tag="a3p", name="a3p")
            gq_all = cp.tile([40, BS], F32, tag="gq_all", name="gq_all")
            s_acc = [cp.tile([1, 400], F32, tag=f"s_acc{i}", name=f"s_acc{i}") for i in range(2)]
            sq_acc = [cp.tile([40, 1], F32, tag=f"sq_acc{i}", name=f"sq_acc{i}") for i in range(2)]
            rq_t = [cp.tile([128, BS], F32, tag=f"rq{t}", name=f"rq{t}") for t in range(NT)]
            ra_t = [cp.tile([128, BS], F32, tag=f"ra{t}", name=f"ra{t}") for t in range(NT)]
            rq3p = cp.tile([128, CHUNK], F32, tag="rq3p", name="rq3p")
            ra3p = cp.tile([128, NG], F32, tag="ra3p", name="ra3p")

            ccin = [dp.tile([1, 440], F32, tag=f"ccin{i}", name=f"ccin{i}") for i in range(2)]
            ccout = [dp.tile([8, 440], F32, tag=f"ccout{i}", name=f"ccout{i}") for i in range(2)]
            ccwi = dp.tile([1, 8], F32, tag="ccwi", name="ccwi")
            ccwo = dp.tile([1, 8], F32, tag="ccwo", name="ccwo")

            # ---- load constants/inputs ----
            _pab_cm = tc.tile_pool(name="pab", bufs=1)
            pab_pool = _pab_cm.__enter__()
            p_sb = [pab_pool.tile([128, MT], BF16, tag=f"p_sb{t}", name=f"p_sb{t}") for t in range(NT)]
            p4r = pab_pool.tile([128, MT], BF16, tag="p4r", name="p4r")
            _xap_cm = tc.tile_pool(name="xa_pool", bufs=PRE)
            xap = _xap_cm.__enter__()
            _pa_cm = tc.tile_pool(name="pa", bufs=1)
            pa_pool = _pa_cm.__enter__()
            xq_all = pa_pool.tile([128, 8 * MT], BF16, tag="xq_all", name="xq_all")
            wq_sb = pa_pool.tile([128, 8 * CP], BF16, tag="wq_sb", name="wq_sb")
            u_sb = pa_pool.tile([128, NT * CP], BF16, tag="u_sb", name="u_sb")
            # phase-A loads split over the sync and scalar queues so the
            # Q matmuls' inputs all land within the first ~12us
            for k in range(4):
                nc.sync.dma_start(wq_sb[:, k * CP:(k + 1) * CP], wq_d[k])
                nc.scalar.dma_start(wq_sb[:, (k + 4) * CP:(k + 5) * CP], wq_d[k + 4])
            for k in range(4):
                nc.sync.dma_start(xq_all[:, k * MT:k * MT + 640], xq_d[k][:, 0:640])
                nc.scalar.dma_start(xq_all[:, (k + 4) * MT:(k + 4) * MT + 640],
                                    xq_d[k + 4][:, 0:640])
            for k in range(4):
                nc.sync.dma_start(xq_all[:, k * MT + 640:(k + 1) * MT],
                                  xq_d[k][:, 640:MT])
                nc.scalar.dma_start(xq_all[:, (k + 4) * MT + 640:(k + 5) * MT],
                                    xq_d[k + 4][:, 640:MT])
            nc.sync.dma_start(bq_sb[:], bq_d[:])
            nc.scalar.dma_start(
                u_sb[:].rearrange("p (k d) -> p k d", k=NT),
                u_d[:].rearrange("k p d -> p k d"))
            nc.scalar.dma_start(
                wa_sb[:].rearrange("p (k c) -> p k c", k=8),
                wa_d[:].rearrange("k p c -> p k c"))
            nc.scalar.dma_start(ba_sb[:], ba_d[:])
            nc.scalar.dma_start(b4_sb[:], b4_d[:])
            nc.scalar.dma_start(u3_sb[:], u3_d[:])
            nc.sync.dma_start(id_sb[:], id_d[:])
            nc.sync.dma_start(on_sb[:], on_d[:])
            nc.sync.dma_start(oc_sb[:], oc_d[:])

            xts = {}

            def fetch(bb, eng=nc.gpsimd):
                xt = xap.tile([128, 8 * 400], BF16, tag="xa_t", name="xa_t")
                for h in range(2):
                    eng.dma_start(
                        xt[:, 1600 * h:1600 * (h + 1)].rearrange("p (k l) -> p k l", k=4),
                        xa_d[bb][4 * h:4 * (h + 1)].rearrange("k p l -> p k l"))
                xts[bb] = xt

            # stagger the prefetch so it doesn't crowd the phase-A loads
            fetch(0)
            fetch(1)

            # warm the ACT LUT table set (exp_and_others)
            warm = cp.tile([1, 32], F32, tag="warm", name="warm")
            nc.vector.memset(warm[:, :], 0.25)
            nc.scalar.activation(warm[:, :], warm[:, :], AF.Tanh)
            nc.scalar.activation(warm[:, :], warm[:, :], AF.Exp)

            # dummy collective: pre-pays the CC mesh wakeup
            wsync = cp.tile([1, 8], F32, tag="wsync", name="wsync")
            nc.vector.memset(wsync[:, :], 1.0)
            nc.gpsimd.dma_start(ccwi[0:1, :], wsync[:, :])
            nc.gpsimd.collective_compute(
                "AllReduce", OP.add,
                replica_groups=[list(range(N_CORES))],
                ins=[ccwi[:].opt()], outs=[ccwo[:].opt()])

            nc.vector.memset(s_acc[0][:, :], 0.0)
            nc.vector.memset(s_acc[1][:, :], 0.0)
            nc.vector.memset(sq_acc[0][:, :], 0.0)
            nc.vector.memset(sq_acc[1][:, :], 0.0)
            nc.vector.memset(rq_t[3][:, :], 0.0)
            nc.vector.memset(ra_t[3][:, :], 0.0)

            # ---- Phase A: Q encode + P = U^T Q ----
            with tc.tile_pool(name="qpsum", bufs=8, space="PSUM") as qp:
                for t in range(3):
                    ps = [qp.tile([128, CHUNK * M], F32, tag="qps", name="qps") for _ in range(NCH)]
                    for k in range(8):
                        lhsT = wq_sb[:, k * CP + t * 128:k * CP + (t + 1) * 128]
                        for s in range(NCH):
                            rhs = xq_all[:, k * MT + s * CHUNK * M:
                                         k * MT + (s + 1) * CHUNK * M]
                            nc.tensor.matmul(ps[s][:, :], lhsT, rhs,
                                             start=(k == 0), stop=(k == 7))
                    for s in range(NCH):
                        nc.vector.tensor_add(
                            q_sb[t][:, s * CHUNK * M:(s + 1) * CHUNK * M],
                            ps[s][:, :],
                            bq_sb[:, t:t + 1].broadcast_to((128, CHUNK * M)))
                    fetch(2 + t)
                # c4 tile: 4 chunks packed into PSUM col strips 0/32/64/96
                qpk = qp.tile([128, CHUNK * M], F32, tag="qps", name="qpk")
                for k in range(8):
                    lhsT = wq_sb[:, k * CP + 384:k * CP + 400]
                    for s in range(NCH):
                        rhs = xq_all[:, k * MT + s * CHUNK * M:
                                     k * MT + (s + 1) * CHUNK * M]
                        nc.tensor.matmul(qpk[32 * s:32 * s + 16, :], lhsT, rhs,
                                         start=(k == 0), stop=(k == 7),
                                         tile_position=(0, 32 * s))
                nc.scalar.activation(q3p[:, :], qpk[:, :], AF.Identity,
                                     bias=b4_sb[:, 0:1])
                fetch(5)

            with tc.tile_pool(name="ppsum", bufs=8, space="PSUM") as pp:
                for t in range(NT):
                    for s in range(NCH):
                        ps = pp.tile([128, CHUNK * M], F32, tag="pps", name="pps")
                        for kk in range(3):
                            lhsT = u_sb[:, kk * CP + t * 128:kk * CP + (t + 1) * 128]
                            rhs = q_sb[kk][:, s * CHUNK * M:(s + 1) * CHUNK * M]
                            nc.tensor.matmul(ps[:, :], lhsT, rhs,
                                             start=(kk == 0), stop=False)
                        # c4 contraction from the strip-packed q3p
                        nc.tensor.matmul(ps[:, :],
                                         u3_sb[32 * s:32 * s + 16, t * 128:(t + 1) * 128],
                                         q3p[32 * s:32 * s + 16, :],
                                         start=False, stop=True,
                                         tile_position=(32 * s, 0))
                        nc.vector.tensor_copy(
                            p_sb[t][:, s * CHUNK * M:(s + 1) * CHUNK * M], ps[:, :])

            _pa_cm.__exit__(None, None, None)

            # replicate P's c4 rows to strips 0/32/64/96 for the packed G kk=3
            for x in range(4):
                nc.sync.dma_start(p4r[32 * x:32 * x + 16, :], p_sb[3][0:16, :])

            # ---- Phase B: per-batch A encode + paired G + maxes + exp ----
            with tc.tile_pool(name="apsum", bufs=4, space="PSUM") as ap, \
                 tc.tile_pool(name="apk", bufs=2, space="PSUM") as apkp, \
                 tc.tile_pool(name="gpsum", bufs=2, space="PSUM") as gp, \
                 tc.tile_pool(name="ebc", bufs=2) as ebcp, \
                 tc.tile_pool(name="rows", bufs=4) as rowp, \
                 tc.tile_pool(name="tree", bufs=4) as trp:

                e_grp = {}
                apk = {}

                g_back_args = {}

                def g_front(bb, g2, off):
                    last = bb >= SPLIT
                    nc.vector.reduce_max(gq_all[0:40, bb:bb + 1], g2[off:off + 40, :],
                                         axis=mybir.AxisListType.X, op=OP.max)
                    e1q = rowp.tile([40, 1], F32, tag="e1q", name="e1q")
                    nc.scalar.activation(e1q[:, :], gq_all[0:40, bb:bb + 1],
                                         AF.Tanh)
                    nc.scalar.activation(e1q[:, :], e1q[:, :], AF.Exp)
                    nc.vector.tensor_add(sq_acc[1 if last else 0][:, :],
                                         sq_acc[1 if last else 0][:, :], e1q[:, :])
                    if last:
                        g_s = g_s4[:, (bb - SPLIT) * 400:(bb - SPLIT + 1) * 400]
                    else:
                        g_s = trp.tile([40, 400], F32, tag="g_s", name="g_s")
                    nc.scalar.activation(g_s[:, :], g2[off:off + 40, :], AF.Copy)
                    g_back_args[bb] = g_s

                def g_back(bb):
                    last = bb >= SPLIT
                    g_s = g_back_args.pop(bb)
                    g_r = trp.tile([40, 400], F32, tag="g_r", name="g_r")
                    nc.gpsimd.partition_all_reduce(
                        g_r[:, :], g_s[:, :], channels=40,
                        reduce_op=bass_isa.ReduceOp.max)
                    t1 = rowp.tile([1, 400], F32, tag="t1", name="t1")
                    nc.scalar.activation(t1[:, :], g_r[0:1, :], AF.Tanh)
                    e1 = rowp.tile([1, 400], F32, tag="e1", name="e1")
                    nc.scalar.activation(e1[:, :], t1[:, :], AF.Exp)
                    if not last:
                        nc.vector.tensor_add(s_acc[0][:, :], s_acc[0][:, :], e1[:, :])
                    e1b = rowp.tile([1, 400], BF16, tag="e1b", name="e1b")
                    nc.vector.tensor_copy(e1b[:, :], e1[:, :])
                    eg = e_grp[bb // GRP]
                    nc.gpsimd.partition_broadcast(
                        eg[:, (bb % GRP) * 400:(bb % GRP + 1) * 400], e1b[:, :])

                def g_chain(bb, g2, off):
                    g_front(bb, g2, off)
                    g_back(bb)

                g_pair_ps = {}

                def do_g_pair_mm(b0):
                    """G matmuls only; g2 psum kept for deferred fronts."""
                    g2 = gp.tile([128, 400], F32, tag="gps", name="gps")
                    for kk in range(3):
                        for j in range(2):
                            bb = b0 + j
                            nc.tensor.matmul(
                                g2[64 * j:64 * j + 40, :],
                                p_sb[kk][:, bb * M:(bb + 1) * M],
                                a_sb[kk][:, bb * L:(bb + 1) * L],
                                start=(kk == 0), stop=False,
                                tile_position=(0, 64 * j))
                    for j in range(2):
                        bb = b0 + j
                        x = bb % GRP
                        g = bb // GRP
                        nc.tensor.matmul(
                            g2[64 * j:64 * j + 40, :],
                            p4r[32 * x:32 * x + 16, bb * M:(bb + 1) * M],
                            a3p[32 * x:32 * x + 16, g * 400:(g + 1) * 400],
                            start=False, stop=(j == 1),
                            tile_position=(32 * x, 64 * j))
                    g_pair_ps[b0] = g2

                def do_g_pair(b0):
                    """G for batches b0, b0+1 packed via col groups 0/64."""
                    g2 = gp.tile([128, 400], F32, tag="gps", name="gps")
                    for kk in range(3):
                        for j in range(2):
                            bb = b0 + j
                            nc.tensor.matmul(
                                g2[64 * j:64 * j + 40, :],
                                p_sb[kk][:, bb * M:(bb + 1) * M],
                                a_sb[kk][:, bb * L:(bb + 1) * L],
                                start=(kk == 0), stop=False,
                                tile_position=(0, 64 * j))
                    for j in range(2):
                        bb = b0 + j
                        x = bb % GRP
                        g = bb // GRP
                        nc.tensor.matmul(
                            g2[64 * j:64 * j + 40, :],
                            p4r[32 * x:32 * x + 16, bb * M:(bb + 1) * M],
                            a3p[32 * x:32 * x + 16, g * 400:(g + 1) * 400],
                            start=False, stop=(j == 1),
                            tile_position=(32 * x, 64 * j))
                    g_chain(b0, g2, 0)
                    g_chain(b0 + 1, g2, 64)

                def y_mult(g):
                    # Y = A .* exp-broadcast over this group's slice
                    eg = e_grp.pop(g)
                    sl = slice(g * GRP * L, (g + 1) * GRP * L)
                    for t in range(3):
                        nc.vector.tensor_tensor(a_sb[t][:, sl], a_sb[t][:, sl],
                                                eg[:, :], op=OP.mult)
                    # strip-packed c4 tile: per-strip slices of eg line up
                    # with the per-batch strips of a3p
                    eg3 = ebcp.tile([128, 400], BF16, tag="eg3", name="eg3")
                    for x in range(GRP):
                        nc.vector.tensor_copy(
                            eg3[32 * x:32 * x + 16, :],
                            eg[32 * x:32 * x + 16, x * 400:(x + 1) * 400])
                    nc.vector.tensor_tensor(a3p[:, g * 400:(g + 1) * 400],
                                            a3p[:, g * 400:(g + 1) * 400],
                                            eg3[:, :], op=OP.mult)

                for bb in range(BS):
                    if bb + PRE < BS:
                        fetch(bb + PRE)
                    g = bb // GRP
                    x = bb % GRP
                    if x == 0:
                        e_grp[g] = ebcp.tile([128, GRP * 400], BF16,
                                             tag="e_g", name="e_g")
                        apk[g] = apkp.tile([128, 400], F32, tag="apk", name="apk")
                    xt = xts[bb]
                    for t in range(3):
                        aps = ap.tile([128, 400], F32, tag="aps", name="aps")
                        for k in range(8):
                            lhsT = wa_sb[:, k * CP + t * 128:k * CP + (t + 1) * 128]
                            nc.tensor.matmul(aps[:, :], lhsT, xt[:, k * 400:(k + 1) * 400],
                                             start=(k == 0), stop=(k == 7))
                        nc.scalar.activation(a_sb[t][:, bb * L:(bb + 1) * L],
                                             aps[:, :], AF.Identity,
                                             bias=ba_sb[:, t:t + 1])
                    if x == GRP - 1:
                        # c4 rows for the whole group: k-outer / strip-inner
                        # so the 4 col-group matmuls of each k overlap in
                        # the PE array
                        pk = apk.pop(g)
                        for k in range(8):
                            lhsT = wa_sb[:, k * CP + 384:k * CP + 400]
                            for x2 in range(GRP):
                                xt2 = xts[bb - 3 + x2]
                                nc.tensor.matmul(pk[32 * x2:32 * x2 + 16, :], lhsT,
                                                 xt2[:, k * 400:(k + 1) * 400],
                                                 start=(k == 0), stop=(k == 7),
                                                 tile_position=(0, 32 * x2))
                        nc.scalar.activation(a3p[:, g * 400:(g + 1) * 400],
                                             pk[:, :], AF.Identity,
                                             bias=b4_sb[:, 1:2])
                        for x2 in range(GRP):
                            xts.pop(bb - 3 + x2)
                        if bb < BS - 1:
                            do_g_pair(bb - 3)
                            do_g_pair(bb - 1)
                            y_mult(g)
                        else:
                            do_g_pair_mm(bb - 3)
                            do_g_pair_mm(bb - 1)
                            for b2 in range(SPLIT, BS):
                                g_front(b2, g_pair_ps[SPLIT + 2 * ((b2 - SPLIT) // 2)],
                                        64 * ((b2 - SPLIT) % 2))
                            # transposed AllGather-2 contribution on the now
                            # idle PE; its trigger goes ahead of the gpsimd
                            # PAR/broadcast tail
                            for q2 in range(4):
                                tpb = gp.tile([128, 400], F32, tag="gps", name="tpb")
                                for j in range(4):
                                    w = 128 if j < 3 else 16
                                    nc.tensor.transpose(
                                        tpb[0:w, 40 * j:40 * j + 40],
                                        g_s4[:, q2 * 400 + 128 * j:q2 * 400 + 128 * j + w],
                                        id_sb[0:40, 0:40])
                                nc.vector.reduce_max(
                                    et_all[:, 4 * q2:4 * q2 + 4],
                                    tpb[:, 0:160].rearrange("p (j m) -> p j m", j=4),
                                    axis=mybir.AxisListType.X, op=OP.max)
                            nc.scalar.activation(et_all[:, :], et_all[:, :], AF.Tanh)
                            nc.scalar.activation(et_all[:, :], et_all[:, :], AF.Exp)
                            et_v = et_all[:].rearrange("p (q j) -> p q j", q=4)
                            nc.vector.tensor_tensor(et_v[:, 0:2, :], et_v[:, 0:2, :],
                                                    et_v[:, 2:4, :], op=OP.add)
                            sT = rowp.tile([128, 4], F32, tag="sT", name="sT")
                            nc.vector.tensor_tensor(
                                sT[:, :].rearrange("p (o j) -> p o j", o=1),
                                et_v[:, 0:1, :], et_v[:, 1:2, :], op=OP.add)
                            nc.sync.dma_start(ccin[1][0:1, 0:40], sq_acc[1][:, :])
                            nc.sync.dma_start(
                                ccin[1][0:1, 40:424].rearrange("o (j i) -> i (o j)", j=3),
                                sT[:, 0:3])
                            nc.sync.dma_start(
                                ccin[1][0:1, 424:440].rearrange("o (j i) -> i (o j)", j=1),
                                sT[0:16, 3:4])
                            nc.gpsimd.collective_compute(
                                "AllGather", OP.bypass,
                                replica_groups=[list(range(N_CORES))],
                                ins=[ccin[1][:].opt()], outs=[ccout[1][:].opt()])
                            for b2 in range(SPLIT, BS):
                                g_back(b2)
                            y_mult(g)
                    if bb == SPLIT - 1:
                        nc.sync.dma_start(ccin[0][0:1, 0:40], sq_acc[0][:, :])
                        nc.sync.dma_start(ccin[0][0:1, 40:440], s_acc[0][:, :])
                        nc.gpsimd.collective_compute(
                            "AllGather", OP.bypass,
                            replica_groups=[list(range(N_CORES))],
                            ins=[ccin[0][:].opt()], outs=[ccout[0][:].opt()])

            _xap_cm.__exit__(None, None, None)
            _pab_cm.__exit__(None, None, None)

            # ---- Phase C: AllGather #2 + 1/S scale + pooled sums ----
            with tc.tile_pool(name="phc", bufs=1) as pc, \
                 tc.tile_pool(name="cpsum", bufs=2, space="PSUM") as cps, \
                 tc.tile_pool(name="cpsum1", bufs=2, space="PSUM") as cp1:
                sg = pc.tile([16, 440], F32, tag="sg", name="sg")
                nc.sync.dma_start(sg[0:8, :], ccout[0][:, :])

                # --- work hidden under the collective: exp_q ---
                tq = pc.tile([40, BS], F32, tag="tq", name="tq")
                nc.scalar.activation(tq[:, :], gq_all[:, :], AF.Tanh)
                e_q = pc.tile([40, BS], F32, tag="e_q", name="e_q")
                nc.scalar.activation(e_q[:, :], tq[:, :], AF.Exp)
                eqt_ps = cp1.tile([BS, 40], F32, tag="c1", name="eqt_ps", bufs=1)
                nc.tensor.transpose(eqt_ps[:, :], e_q[:, :], id_sb[0:40, 0:40])
                eqt = pc.tile([BS, 40], BF16, tag="eqt", name="eqt")
                nc.vector.tensor_copy(eqt[:, :], eqt_ps[:, :])
                eq_fl = pc.tile([1, MT], BF16, tag="eq_fl", name="eq_fl")
                nc.gpsimd.dma_start(eq_fl[0:1, :], eqt[:, :])
                eq_bc = pc.tile([128, MT], BF16, tag="eq_bc", name="eq_bc")
                for ch in range(0, MT, 512):
                    ce = min(ch + 512, MT)
                    wqb = cps.tile([128, 512], F32, tag="wqb", name="wqb", bufs=1)
                    nc.tensor.matmul(wqb[:, 0:ce - ch], on_sb[:, :],
                                     eq_fl[0:1, ch:ce], start=True, stop=True)
                    nc.scalar.activation(eq_bc[:, ch:ce], wqb[:, 0:ce - ch], AF.Copy)
                for t in range(3):
                    nc.vector.tensor_tensor(q_sb[t][:, :], q_sb[t][:, :],
                                            eq_bc[:, :], op=OP.mult)
                eq3 = pc.tile([128, CHUNK * M], BF16, tag="eq3", name="eq3")
                for sx in range(NCH):
                    nc.vector.tensor_copy(
                        eq3[32 * sx:32 * sx + 16, :],
                        eq_bc[32 * sx:32 * sx + 16, sx * CHUNK * M:(sx + 1) * CHUNK * M])
                nc.vector.tensor_tensor(q3p[:, :], q3p[:, :], eq3[:, :], op=OP.mult)

                # --- post-collective: u = 1/S ---
                nc.sync.dma_start(sg[8:16, :], ccout[1][:, :])
                sgp = cp1.tile([1, 440], F32, tag="sgp", name="sgp", bufs=1)
                nc.tensor.matmul(sgp[:, :], oc_sb[0:16, 0:1], sg[:, :],
                                 start=True, stop=True)
                ur = pc.tile([1, 440], F32, tag="ur", name="ur")
                nc.vector.reciprocal_approx_fast(ur[:, :], sgp[0:1, :])
                ub = pc.tile([1, 440], BF16, tag="ub", name="ub")
                nc.vector.tensor_copy(ub[:, :], ur[:, :])
                ua_bc = pc.tile([128, 400], BF16, tag="ua_bc", name="ua_bc")
                nc.gpsimd.partition_broadcast(ua_bc[:, :], ub[0:1, 40:440])

                uq_bc = pc.tile([128, 40], BF16, tag="uq_bc", name="uq_bc")
                nc.gpsimd.partition_broadcast(uq_bc[:, :], ub[0:1, 0:40])
                ua_v = ua_bc[:].rearrange("p (o l) -> p o l", o=1).broadcast_to((128, BS, 400))
                ua_v8 = ua_bc[:].rearrange("p (o l) -> p o l", o=1).broadcast_to((128, NG, 400))
                uq_v = uq_bc[:].rearrange("p (o m) -> p o m", o=1).broadcast_to((128, BS, 40))
                uq_v8 = uq_bc[:].rearrange("p (o m) -> p o m", o=1).broadcast_to((128, CHUNK, 40))

                def tree_sum(av, out, n, nb):
                    while n > 25 and n % 2 == 0:
                        h = n // 2
                        nc.vector.tensor_tensor(av[:, 0:nb, 0:h], av[:, 0:nb, 0:h],
                                                av[:, 0:nb, h:n], op=OP.add)
                        n = h
                    nc.vector.reduce_sum(out, av[:, 0:nb, 0:n],
                                         axis=mybir.AxisListType.X, op=OP.add)

                # c4 tile first: small, unblocks the repack DMA early
                a3v = a3p[:].rearrange("p (g l) -> p g l", g=NG)
                nc.vector.tensor_tensor(a3v, a3v, ua_v8, op=OP.mult)
                tree_sum(a3v, ra3p[:, :], 400, NG)
                # repack [strip x | c16, g] -> [c16, b=4g+x]
                for x in range(4):
                    nc.sync.dma_start(
                        ra_t[3][0:16, :].rearrange("i (g x) -> i x g", x=4)[:, x:x + 1, :],
                        ra3p[32 * x:32 * x + 16, :].rearrange("i (o g) -> i o g", o=1))
                scr = pc.tile([128, 400], BF16, tag="scr", name="scr")
                for t in range(3):
                    av = a_sb[t][:].rearrange("p (b l) -> p b l", b=BS)
                    nc.vector.tensor_tensor(av, av, ua_v, op=OP.mult)
                    if t == 0:
                        # otherwise-idle scalar engine reduces one c-tile
                        for bb in range(BS):
                            nc.scalar.activation(
                                scr[:, :], av[:, bb, :], AF.Copy,
                                accum_out=ra_t[t][:, bb:bb + 1])
                    elif t == 1:
                        for bb in range(12):
                            nc.scalar.activation(
                                scr[:, :], av[:, bb, :], AF.Copy,
                                accum_out=ra_t[t][:, bb:bb + 1])
                        tv = av[:, 12:BS, :]
                        n = 400
                        while n > 25 and n % 2 == 0:
                            h = n // 2
                            nc.vector.tensor_tensor(tv[:, :, 0:h], tv[:, :, 0:h],
                                                    tv[:, :, h:n], op=OP.add)
                            n = h
                        nc.vector.reduce_sum(ra_t[t][:, 12:BS], tv[:, :, 0:n],
                                             axis=mybir.AxisListType.X, op=OP.add)
                    else:
                        tree_sum(av, ra_t[t][:, :], 400, BS)
                q3v = q3p[:].rearrange("p (j m) -> p j m", j=CHUNK)
                nc.vector.tensor_tensor(q3v, q3v, uq_v8, op=OP.mult)
                tree_sum(q3v, rq3p[:, :], 40, CHUNK)
                for x in range(4):
                    nc.sync.dma_start(
                        rq_t[3][0:16, :].rearrange("i (s j) -> i s j", s=4)[:, x:x + 1, :],
                        rq3p[32 * x:32 * x + 16, :].rearrange("i (o j) -> i o j", o=1))

                for t in range(3):
                    qv = q_sb[t][:].rearrange("p (b m) -> p b m", b=BS)
                    nc.vector.tensor_tensor(qv, qv, uq_v, op=OP.mult)
                    tree_sum(qv, rq_t[t][:, :], 40, BS)

                # cosine similarity via accumulating PE transposes
                def psum_all(tiles, tag):
                    tps = cps.tile([BS, 128], F32, tag="cts", name=f"{tag}tp", bufs=2)
                    for t in range(NT):
                        nc.tensor.matmul(tps[:, :], tiles[t][:, :], id_sb[:, :],
                                         is_transpose=True,
                                         start=(t == 0), stop=(t == NT - 1))
                    col = pc.tile([32, 1], F32, tag=f"{tag}c", name=f"{tag}c")
                    nc.vector.reduce_sum(col[:, :], tps[:, :],
                                         axis=mybir.AxisListType.X, op=OP.add)
                    return col

                pr = [pc.tile([128, BS], F32, tag=f"pr{t}", name=f"pr{t}") for t in range(NT)]
                pq = [pc.tile([128, BS], F32, tag=f"pq{t}", name=f"pq{t}") for t in range(NT)]
                pa = [pc.tile([128, BS], F32, tag=f"pa{t}", name=f"pa{t}") for t in range(NT)]
                for t in range(NT):
                    nc.vector.tensor_mul(pr[t][:, :], rq_t[t][:, :], ra_t[t][:, :])
                    nc.vector.tensor_mul(pq[t][:, :], rq_t[t][:, :], rq_t[t][:, :])
                    nc.vector.tensor_mul(pa[t][:, :], ra_t[t][:, :], ra_t[t][:, :])
                dot = psum_all(pr, "dt")
                qq = psum_all(pq, "qq")
                aa = psum_all(pa, "aa")

                nq = pc.tile([32, 1], F32, tag="nq", name="nq")
                na = pc.tile([32, 1], F32, tag="na", name="na")
                nc.scalar.activation(nq[:, :], qq[:, :], AF.Sqrt)
                nc.scalar.activation(na[:, :], aa[:, :], AF.Sqrt)
                nc.vector.tensor_scalar_max(nq[:, :], nq[:, :], 1e-6)
                nc.vector.tensor_scalar_max(na[:, :], na[:, :], 1e-6)
                den = pc.tile([32, 1], F32, tag="den", name="den")
                nc.vector.tensor_mul(den[:, :], nq[:, :], na[:, :])
                rden = pc.tile([32, 1], F32, tag="rden", name="rden")
                nc.vector.reciprocal(rden[:, :], den[:, :])
                res = pc.tile([32, 1], F32, tag="res", name="res")
                nc.vector.tensor_mul(res[:, :], dot[:, :], rden[:, :])
                nc.gpsimd.dma_start(out_d[:].rearrange("(a b) -> a b", b=1),
                                    res[:, :])

    nc.finalize()
    return nc


def _prep(question, answer, Wq, bq, Wa, ba, U):
    bf = ml_dtypes.bfloat16
    qs = question.reshape(N_CORES, BS, M, E)
    as_ = answer.reshape(N_CORES, BS, L, E)

    def enc_z8(x, T):
        # x: [BS, T, E] -> Z^T rows [BS, 8, 128, T] bf16 (ctx shifts baked in)
        xt = x.transpose(0, 2, 1)  # [BS, E, T]
        xtp = np.zeros((x.shape[0], E, T + 2), np.float32)
        xtp[:, :, 1:T + 1] = xt
        z = np.zeros((x.shape[0], 1024, T), dtype=bf)
        for i in range(3):
            z[:, i * E:(i + 1) * E, :] = xtp[:, :, i:i + T].astype(bf)
        return z.reshape(x.shape[0], 8, 128, T)

    def enc_xq8(x):
        # [BS, M, E] -> [8, 128, BS*M] bf16
        z = enc_z8(x, M)  # [BS, 8, 128, 40]
        return np.ascontiguousarray(z.transpose(1, 2, 0, 3)).reshape(8, 128, MT)

    def enc_w8(W):
        # W [C, 900] -> W^T padded [8, 128, CP] bf16
        o = np.zeros((1024, CP), dtype=bf)
        o[0:900, 0:C] = W.T.astype(bf)
        return o.reshape(8, 128, CP)

    up = np.zeros((CP, CP), dtype=bf)
    up[0:C, 0:C] = U.astype(bf)
    up = up.reshape(NT, 128, CP)

    # U rows 384:400 replicated at partition strips 0/32/64/96
    u3r = np.zeros((128, CP), dtype=bf)
    for x in range(4):
        u3r[32 * x:32 * x + 16, 0:C] = U[384:400, :].astype(bf)

    def enc_b(b):
        o = np.zeros((CP,), np.float32)
        o[0:C] = b
        return np.ascontiguousarray(o.reshape(NT, 128).T)

    # bias rows 384:400 replicated at strips, for the packed c4 drains
    b4r = np.zeros((128, 2), np.float32)
    for x in range(4):
        b4r[32 * x:32 * x + 16, 0] = bq[384:400]
        b4r[32 * x:32 * x + 16, 1] = ba[384:400]

    com = {
        "wqt": enc_w8(Wq), "wat": enc_w8(Wa), "ut": up, "u3r": u3r,
        "bq": enc_b(bq), "ba": enc_b(ba), "b4r": b4r,
        "ident": np.eye(128, dtype=np.float32),
        "ones": np.ones((1, 128), dtype=bf),
        "ones_col": np.ones((128, 1), np.float32),
    }
    maps = []
    for i in range(N_CORES):
        m = dict(com)
        m["xq"] = enc_xq8(qs[i])
        m["xa"] = enc_z8(as_[i], L)
        maps.append(m)
    return maps


def kernel(question, answer, Wq, bq, Wa, ba, U, _trace=False):
    if "nc" not in _CACHE:
        _CACHE["nc"] = _build()
    nc = _CACHE["nc"]
    maps = _prep(np.asarray(question), np.asarray(answer), np.asarray(Wq),
                 np.asarray(bq), np.asarray(Wa), np.asarray(ba), np.asarray(U))
    r = run_bass_kernel_spmd(nc, maps, list(range(N_CORES)), trace=_trace)
    _CACHE["last"] = r
    return np.concatenate([r.results[i]["out"] for i in range(N_CORES)])
